# revision 5
# baseline (speedup 1.0000x reference)
"""Trainium2 Bass kernel for causal self-attention with RoPE.

Problem shapes (hardcoded): B=2, L=2048, D=1024, N=16 heads, H=64.

Sharding (8 cores, fully collective-free): data-parallel over batch
(2 groups of 4 cores), tensor-parallel over heads within a group
(4 heads/core).  Each core:
  1. computes q,k for its 4 heads in h-major layout (transposed matmul
     orientation: lhsT = w columns, rhs = x^T), applies RoPE on-chip,
  2. computes v in L-major layout (normal orientation),
  3. runs causal flash-style attention with scores transposed
     (S^T[key, query]) so softmax sums ride a fused ones-column through
     the PV matmul (no transposes anywhere),
  4. computes its PARTIAL output projection: its 4 heads' attention
     outputs (256 contraction dims) times the matching 256-row slice of
     w_proj, giving a full [L, D] partial in bf16.
Host code reformats/shards inputs (transpose, bf16 cast, column
permutation, table replication) and sums the 4 partials per batch.
There is no cross-core communication anywhere, so each core's
execution span is independent of peer launch skew.
"""

import numpy as np
import ml_dtypes

B, L, D, N_HEADS, H = 2, 2048, 1024, 16, 64
HPC = 4          # heads per core
GROUP = 4        # cores per batch group
NCORES = 8
QT = 512         # query tile width (matmul free dim)
KB = 128         # key block (psum partition dim)
N_QT = L // QT   # 4 query tiles
N_DC = D // 128  # 8 contraction chunks
N_LC = L // 128  # 16 L chunks for v / output rows
WPR = HPC * H    # w_proj rows per core (256)
BF16 = ml_dtypes.bfloat16

_prog_cache = {}


def _build_program():
    if "nc" in _prog_cache:
        return _prog_cache["nc"]

    import concourse.bass as bass
    import concourse.mybir as mybir
    import concourse.tile as tile
    from concourse import bacc
    from contextlib import ExitStack

    bf = mybir.dt.bfloat16
    f32 = mybir.dt.float32

    nc = bacc.Bacc(num_devices=NCORES)

    xt = nc.dram_tensor("xt", [D, L], bf, kind="ExternalInput")
    wqk = nc.dram_tensor("wqk", [D, 2 * HPC * H], bf, kind="ExternalInput")
    wv = nc.dram_tensor("wv", [D, HPC * H], bf, kind="ExternalInput")
    wp = nc.dram_tensor("wp", [WPR, D], bf, kind="ExternalInput")
    ctab = nc.dram_tensor("ctab", [128, L], bf, kind="ExternalInput")
    stab = nc.dram_tensor("stab", [128, L], bf, kind="ExternalInput")
    tri = nc.dram_tensor("tri", [128, 128], bf, kind="ExternalInput")
    out = nc.dram_tensor("out", [L, D], bf, kind="ExternalOutput")
    zdram = nc.dram_tensor("zdram", [HPC * N_QT, QT], f32, kind="Internal")

    Exp = mybir.ActivationFunctionType.Exp
    Copy = mybir.ActivationFunctionType.Copy
    SCALE = 1.0 / 8.0  # 1/sqrt(H)

    with tile.TileContext(nc) as tc, ExitStack() as ctx:
        singles = ctx.enter_context(tc.tile_pool(name="singles", bufs=1))
        work = ctx.enter_context(tc.tile_pool(name="work", bufs=3))
        epool = ctx.enter_context(tc.tile_pool(name="epool", bufs=3))
        dpool = ctx.enter_context(tc.tile_pool(name="dpool", bufs=2))
        opool = ctx.enter_context(tc.tile_pool(name="opool", bufs=2))
        ps_scores = ctx.enter_context(
            tc.tile_pool(name="ps_scores", bufs=2, space="PSUM")
        )
        ps_pv = ctx.enter_context(tc.tile_pool(name="ps_pv", bufs=2, space="PSUM"))
        ps_proj = ctx.enter_context(
            tc.tile_pool(name="ps_proj", bufs=2, space="PSUM")
        )

        # ---- load inputs to SBUF ----
        xt_sb = singles.tile([128, N_DC, L], bf)
        for dc in range(N_DC):
            nc.sync.dma_start(
                out=xt_sb[:, dc, :], in_=xt[128 * dc : 128 * (dc + 1), :]
            )
        wqk_sb = singles.tile([128, N_DC, 4, 128], bf)
        for dc in range(N_DC):
            nc.sync.dma_start(
                out=wqk_sb[:, dc, :, :],
                in_=wqk[128 * dc : 128 * (dc + 1), :].rearrange(
                    "p (qc m) -> p qc m", qc=4
                ),
            )
        wv_sb = singles.tile([128, N_DC, HPC * H], bf)
        for dc in range(N_DC):
            nc.sync.dma_start(
                out=wv_sb[:, dc, :], in_=wv[128 * dc : 128 * (dc + 1), :]
            )
        # w_proj rows for this core's 4 heads: chunk j holds rows for
        # heads (2j, 2j+1) stacked across the 128 partitions
        wp_sb = singles.tile([128, 2, D], bf)
        for j in range(2):
            nc.sync.dma_start(
                out=wp_sb[:, j, :], in_=wp[128 * j : 128 * (j + 1), :]
            )
        ctab_sb = singles.tile([128, L], bf)
        stab_sb = singles.tile([128, L], bf)
        tri_sb = singles.tile([128, 128], bf)
        nc.gpsimd.dma_start(out=ctab_sb, in_=ctab[:, :])
        nc.gpsimd.dma_start(out=stab_sb, in_=stab[:, :])
        nc.gpsimd.dma_start(out=tri_sb, in_=tri[:, :])

        # ---- q,k projection (transposed orientation) + RoPE ----
        # qk chunks: 0,1 = q heads (0,1),(2,3); 2,3 = k heads (0,1),(2,3)
        qk_roped = singles.tile([128, 4, L], bf)
        for qc in range(4):
            for lt in range(N_QT):
                lsl = slice(QT * lt, QT * (lt + 1))
                ps = ps_proj.tile([128, QT], f32, tag="proj")
                for dc in range(N_DC):
                    nc.tensor.matmul(
                        ps,
                        lhsT=wqk_sb[:, dc, qc, :],
                        rhs=xt_sb[:, dc, lsl],
                        start=(dc == 0),
                        stop=(dc == N_DC - 1),
                    )
                qk_bf = work.tile([128, QT], bf, tag="qkbf")
                nc.scalar.activation(out=qk_bf, in_=ps, func=Copy)
                # rot[p] = qk_bf[p ^ 1]  (adjacent even/odd partner swap,
                # a within-32-partition permutation -> stream_shuffle)
                rot = work.tile([128, QT], bf, tag="rot")
                nc.vector.stream_shuffle(
                    rot, qk_bf, mask=[i ^ 1 for i in range(32)]
                )
                m1 = work.tile([128, QT], bf, tag="m1")
                nc.vector.tensor_mul(m1, qk_bf, ctab_sb[:, lsl])
                m2 = work.tile([128, QT], bf, tag="m2")
                nc.vector.tensor_mul(m2, rot, stab_sb[:, lsl])
                nc.vector.tensor_add(qk_roped[:, qc, lsl], m1, m2)

        # ---- v projection (normal orientation), with ones column fused ----
        # per L-chunk layout: [v_h0(64) 1 | v_h1(64) 1 | v_h2(64) 1 | v_h3(64) 1]
        v_sb = singles.tile([128, N_LC, HPC * (H + 1)], bf)
        for h in range(HPC):
            nc.vector.memset(v_sb[:, :, (H + 1) * h + H], 1.0)
        for lc in range(N_LC):
            ps = ps_proj.tile([128, HPC * H], f32, tag="proj")
            for dc in range(N_DC):
                nc.tensor.matmul(
                    ps,
                    lhsT=xt_sb[:, dc, 128 * lc : 128 * (lc + 1)],
                    rhs=wv_sb[:, dc, :],
                    start=(dc == 0),
                    stop=(dc == N_DC - 1),
                )
            vstage = work.tile([128, HPC * H], bf, tag="vstage")
            nc.vector.tensor_copy(vstage, ps)
            for h in range(HPC):
                nc.vector.tensor_copy(
                    v_sb[:, lc, (H + 1) * h : (H + 1) * h + H],
                    vstage[:, H * h : H * (h + 1)],
                )

        # ---- attention (scores transposed; 2-key-block groups) ----
        # attn_all chunk j holds heads (2j, 2j+1) on partition halves,
        # matching the wp_sb row layout for the final contraction.
        attn_all = singles.tile([128, 2, L], bf)
        for h in range(HPC):
            qc = h // 2
            kc = 2 + h // 2
            base = 64 * (h % 2)
            q_all = qk_roped[base : base + 64, qc, :]
            k_all = qk_roped[base : base + 64, kc, :]
            for t in range(N_QT):
                qsl = slice(QT * t, QT * (t + 1))
                po = ps_pv.tile([H + 1, QT], f32, tag="pv")
                n_kb = 4 * (t + 1)
                for g in range(n_kb // 2):
                    pss = ps_scores.tile([128, 2 * QT], f32, tag="scores")
                    et = epool.tile([128, 2 * QT], bf, tag="etile")
                    for j in range(2):
                        kb = 2 * g + j
                        d = 128 * kb - QT * t  # kb/qt diagonal offset
                        lo = max(d, 0)
                        nc.tensor.matmul(
                            pss[:, QT * j + lo : QT * (j + 1)],
                            lhsT=k_all[:, 128 * kb : 128 * (kb + 1)],
                            rhs=q_all[:, QT * t + lo : QT * (t + 1)],
                            start=True,
                            stop=True,
                        )
                    # exp (with 1/sqrt(H) scale); diag blocks get separate
                    # calls restricted to their valid column range
                    if 128 * (2 * g + 1) - QT * t < 0:
                        nc.scalar.activation(
                            out=et, in_=pss, func=Exp, scale=SCALE
                        )
                    else:
                        for j in range(2):
                            kb = 2 * g + j
                            lo = max(128 * kb - QT * t, 0)
                            nc.scalar.activation(
                                out=et[:, QT * j + lo : QT * (j + 1)],
                                in_=pss[:, QT * j + lo : QT * (j + 1)],
                                func=Exp,
                                scale=SCALE,
                            )
                    for j in range(2):
                        kb = 2 * g + j
                        d = 128 * kb - QT * t
                        lo = max(d, 0)
                        if d >= -127:
                            # boundary block: zero strictly-masked entries
                            nc.vector.tensor_mul(
                                et[:, QT * j + lo : QT * j + lo + 128],
                                et[:, QT * j + lo : QT * j + lo + 128],
                                tri_sb,
                            )
                        nc.tensor.matmul(
                            po[:, lo:QT],
                            lhsT=v_sb[:, kb, (H + 1) * h : (H + 1) * (h + 1)],
                            rhs=et[:, QT * j + lo : QT * (j + 1)],
                            start=(kb == 0),
                            stop=(kb == n_kb - 1),
                        )
                # normalize: attn = po[0:64] * (1 / po[64])  broadcast via a
                # DRAM round-trip (write the 1/z row, read it back with a
                # stride-0 partition AP across 64 partitions)
                import concourse.bass as bass

                zs = dpool.tile([H + 1, QT], f32, tag="zs")
                nc.scalar.activation(
                    out=zs[H : H + 1, :], in_=po[H : H + 1, :], func=Copy
                )
                rs = dpool.tile([H + 1, QT], f32, tag="rs")
                nc.vector.reciprocal(
                    out=rs[H : H + 1, :], in_=zs[H : H + 1, :]
                )
                zslot = zdram[N_QT * h + t : N_QT * h + t + 1, :]
                nc.sync.dma_start(out=zslot, in_=rs[H : H + 1, :])
                rb = dpool.tile([H, QT], f32, tag="rb")
                nc.sync.dma_start(
                    out=rb,
                    in_=bass.AP(
                        tensor=zslot.tensor, offset=zslot.offset,
                        ap=[[0, H]] + zslot.ap[1:],
                    ),
                )
                if h % 2 == 0:
                    nc.vector.tensor_mul(
                        attn_all[0:H, h // 2, qsl], po[0:H, :], rb
                    )
                else:
                    # DVE lanes are partition-aligned; route the odd head
                    # to partitions 64..127 with a local SBUF->SBUF DMA
                    attn_sb = dpool.tile([H, QT], bf, tag="attn")
                    nc.vector.tensor_mul(attn_sb, po[0:H, :], rb)
                    nc.sync.dma_start(
                        out=attn_all[64 : 64 + H, h // 2, qsl], in_=attn_sb
                    )

        # ---- partial output projection (full [L, D], contraction over
        # this core's 4 heads = 2 chunks of 128 partitions) ----
        for lc in range(N_LC):
            lsl = slice(128 * lc, 128 * (lc + 1))
            for oc in range(2):
                osl = slice(QT * oc, QT * (oc + 1))
                ps = ps_proj.tile([128, QT], f32, tag="proj")
                for j in range(2):
                    nc.tensor.matmul(
                        ps,
                        lhsT=attn_all[:, j, lsl],
                        rhs=wp_sb[:, j, osl],
                        start=(j == 0),
                        stop=(j == 1),
                    )
                osb = opool.tile([128, QT], bf, tag="osb")
                nc.vector.tensor_copy(osb, ps)
                nc.sync.dma_start(out=out[lsl, osl], in_=osb)

    nc.compile()
    _prog_cache["nc"] = nc
    return nc


def _host_inputs(x, rope, w_qkv, w_proj):
    """Shard + reformat the full inputs for the 8 cores."""
    rope = np.asarray(rope, dtype=np.float32)
    x = np.asarray(x, dtype=np.float32)
    w_qkv = np.asarray(w_qkv, dtype=np.float32)
    w_proj = np.asarray(w_proj, dtype=np.float32)

    xt_b = [np.ascontiguousarray(x[b].T).astype(BF16) for b in range(B)]

    # rope tables in h-major chunk layout: partition p of a 2-head chunk is
    # head (p // 64), component (p % 64); pair index i = (p % 64) // 2
    i_of_p = (np.arange(128) % 64) // 2
    cos_li = rope[:, :, 0]  # (L, 32)
    sin_li = rope[:, :, 1]
    ctab = np.ascontiguousarray(cos_li[:, i_of_p].T).astype(BF16)
    sign = np.where(np.arange(128) % 2 == 0, -1.0, 1.0).astype(np.float32)
    stab = np.ascontiguousarray((sin_li[:, i_of_p] * sign[None, :]).T).astype(BF16)

    # tri[p, f] = 1 where key offset p <= query offset f (keep), else 0
    tri = (np.arange(128)[:, None] <= np.arange(128)[None, :]).astype(BF16)

    in_maps = []
    for c in range(NCORES):
        b, g = divmod(c, GROUP)
        heads = [HPC * g + i for i in range(HPC)]
        wq = np.concatenate([w_qkv[:, H * n : H * (n + 1)] for n in heads], 1)
        wk = np.concatenate(
            [w_qkv[:, D + H * n : D + H * (n + 1)] for n in heads], 1
        )
        wvv = np.concatenate(
            [w_qkv[:, 2 * D + H * n : 2 * D + H * (n + 1)] for n in heads], 1
        )
        in_maps.append(
            {
                "xt": xt_b[b],
                "wqk": np.ascontiguousarray(
                    np.concatenate([wq, wk], 1)
                ).astype(BF16),
                "wv": np.ascontiguousarray(wvv).astype(BF16),
                "wp": np.ascontiguousarray(
                    w_proj[WPR * g : WPR * (g + 1), :]
                ).astype(BF16),
                "ctab": ctab,
                "stab": stab,
                "tri": tri,
            }
        )
    return in_maps


def kernel(x, rope, mask, w_qkv, w_proj, _trace=False):
    from concourse.bass_utils import run_bass_kernel_spmd

    nc = _build_program()
    in_maps = _host_inputs(x, rope, w_qkv, w_proj)
    res = run_bass_kernel_spmd(
        nc, in_maps, core_ids=list(range(NCORES)), trace=_trace
    )
    _prog_cache["last_result"] = res

    full = np.empty((B, L, D), dtype=np.float32)
    for b in range(B):
        acc = np.zeros((L, D), dtype=np.float32)
        for g in range(GROUP):
            acc += np.asarray(res.results[GROUP * b + g]["out"], dtype=np.float32)
        full[b] = acc
    return full


# revision 9
# speedup vs baseline: 1.2477x; 1.2477x over previous
"""Trainium2 Bass kernel for causal self-attention with RoPE.

Problem shapes (hardcoded): B=2, L=2048, D=1024, N=16 heads, H=64.

Sharding (8 cores, fully collective-free): data-parallel over batch
(2 groups of 4 cores), tensor-parallel over heads within a group
(4 heads/core).  Each core:
  1. computes q,k for its 4 heads in h-major layout (transposed matmul
     orientation: lhsT = w columns, rhs = x^T), applies RoPE on-chip,
  2. computes v in L-major layout (normal orientation),
  3. runs causal flash-style attention with scores transposed
     (S^T[key, query]) so softmax sums ride a fused ones-column through
     the PV matmul (no transposes anywhere),
  4. computes its PARTIAL output projection: its 4 heads' attention
     outputs (256 contraction dims) times the matching 256-row slice of
     w_proj, giving a full [L, D] partial in bf16.
Host code reformats/shards inputs (transpose, bf16 cast, column
permutation, table replication) and sums the 4 partials per batch.
There is no cross-core communication anywhere, so each core's
execution span is independent of peer launch skew.
"""

import numpy as np
import ml_dtypes

B, L, D, N_HEADS, H = 2, 2048, 1024, 16, 64
HPC = 4          # heads per core
GROUP = 4        # cores per batch group
NCORES = 8
QT = 512         # query tile width (matmul free dim)
KB = 128         # key block (psum partition dim)
N_QT = L // QT   # 4 query tiles
N_DC = D // 128  # 8 contraction chunks
N_LC = L // 128  # 16 L chunks for v / output rows
WPR = HPC * H    # w_proj rows per core (256)
BF16 = ml_dtypes.bfloat16

_prog_cache = {}


def _build_program():
    if "nc" in _prog_cache:
        return _prog_cache["nc"]

    import concourse.bass as bass
    import concourse.mybir as mybir
    import concourse.tile as tile
    from concourse import bacc
    from contextlib import ExitStack

    bf = mybir.dt.bfloat16
    f32 = mybir.dt.float32

    nc = bacc.Bacc(num_devices=NCORES)

    xt = nc.dram_tensor("xt", [D, L], bf, kind="ExternalInput")
    wqk = nc.dram_tensor("wqk", [D, 2 * HPC * H], bf, kind="ExternalInput")
    wv = nc.dram_tensor("wv", [D, HPC * H], bf, kind="ExternalInput")
    wp = nc.dram_tensor("wp", [WPR, D], bf, kind="ExternalInput")
    ctab = nc.dram_tensor("ctab", [128, L], bf, kind="ExternalInput")
    stab = nc.dram_tensor("stab", [128, L], bf, kind="ExternalInput")
    tri = nc.dram_tensor("tri", [128, 128], bf, kind="ExternalInput")
    out = nc.dram_tensor("out", [L, D], bf, kind="ExternalOutput")

    Exp = mybir.ActivationFunctionType.Exp
    Copy = mybir.ActivationFunctionType.Copy
    SCALE = 1.0 / 8.0  # 1/sqrt(H)

    with tile.TileContext(nc) as tc, ExitStack() as ctx:
        singles = ctx.enter_context(tc.tile_pool(name="singles", bufs=1))
        work = ctx.enter_context(tc.tile_pool(name="work", bufs=3))
        epool = ctx.enter_context(tc.tile_pool(name="epool", bufs=3))
        dpool = ctx.enter_context(tc.tile_pool(name="dpool", bufs=2))
        opool = ctx.enter_context(tc.tile_pool(name="opool", bufs=2))
        ps_scores = ctx.enter_context(
            tc.tile_pool(name="ps_scores", bufs=2, space="PSUM")
        )
        ps_pv = ctx.enter_context(tc.tile_pool(name="ps_pv", bufs=2, space="PSUM"))
        ps_proj = ctx.enter_context(
            tc.tile_pool(name="ps_proj", bufs=2, space="PSUM")
        )

        # ---- load inputs to SBUF (split across sync + pool queues so
        # dispatch overhead parallelizes; xt/wqk interleaved so the first
        # q,k matmul chain can start after the first chunk pair lands) ----
        xt_sb = singles.tile([128, N_DC, L], bf)
        wqk_sb = singles.tile([128, N_DC, 4, 128], bf)
        wv_sb = singles.tile([128, N_DC, HPC * H], bf)
        wp_sb = singles.tile([128, 2, D], bf)
        ctab_sb = singles.tile([128, L], bf)
        stab_sb = singles.tile([128, L], bf)
        tri_sb = singles.tile([128, 128], bf)
        for dc in range(N_DC):
            nc.sync.dma_start(
                out=xt_sb[:, dc, :], in_=xt[128 * dc : 128 * (dc + 1), :]
            )
            nc.gpsimd.dma_start(
                out=wqk_sb[:, dc, :, :],
                in_=wqk[128 * dc : 128 * (dc + 1), :].rearrange(
                    "p (qc m) -> p qc m", qc=4
                ),
            )
        nc.gpsimd.dma_start(out=ctab_sb, in_=ctab[:, :])
        nc.gpsimd.dma_start(out=stab_sb, in_=stab[:, :])
        nc.gpsimd.dma_start(out=tri_sb, in_=tri[:, :])
        for dc in range(N_DC):
            nc.sync.dma_start(
                out=wv_sb[:, dc, :], in_=wv[128 * dc : 128 * (dc + 1), :]
            )
        # w_proj rows for this core's 4 heads: chunk j holds rows for
        # heads (2j, 2j+1) stacked across the 128 partitions
        for j in range(2):
            nc.gpsimd.dma_start(
                out=wp_sb[:, j, :], in_=wp[128 * j : 128 * (j + 1), :]
            )

        # ---- q,k projection (transposed orientation) + RoPE ----
        # qk chunks: 0,1 = q heads (0,1),(2,3); 2,3 = k heads (0,1),(2,3)
        qk_roped = singles.tile([128, 4, L], bf)
        for qc in range(4):
            for lt in range(N_QT):
                lsl = slice(QT * lt, QT * (lt + 1))
                ps = ps_proj.tile([128, QT], f32, tag="proj")
                for dc in range(N_DC):
                    nc.tensor.matmul(
                        ps,
                        lhsT=wqk_sb[:, dc, qc, :],
                        rhs=xt_sb[:, dc, lsl],
                        start=(dc == 0),
                        stop=(dc == N_DC - 1),
                    )
                qk_bf = work.tile([128, QT], bf, tag="qkbf")
                nc.scalar.activation(out=qk_bf, in_=ps, func=Copy)
                # rot[p] = qk_bf[p ^ 1]  (adjacent even/odd partner swap,
                # a within-32-partition permutation -> stream_shuffle)
                rot = work.tile([128, QT], bf, tag="rot")
                nc.vector.stream_shuffle(
                    rot, qk_bf, mask=[i ^ 1 for i in range(32)]
                )
                m1 = work.tile([128, QT], bf, tag="m1")
                nc.vector.tensor_mul(m1, qk_bf, ctab_sb[:, lsl])
                m2 = work.tile([128, QT], bf, tag="m2")
                nc.vector.tensor_mul(m2, rot, stab_sb[:, lsl])
                nc.vector.tensor_add(qk_roped[:, qc, lsl], m1, m2)

        # ---- v projection (normal orientation), with ones column fused ----
        # per L-chunk layout: [v_h0(64) 1 | v_h1(64) 1 | v_h2(64) 1 | v_h3(64) 1]
        v_sb = singles.tile([128, N_LC, HPC * (H + 1)], bf)
        for h in range(HPC):
            nc.vector.memset(v_sb[:, :, (H + 1) * h + H], 1.0)
        for lc in range(N_LC):
            ps = ps_proj.tile([128, HPC * H], f32, tag="proj")
            for dc in range(N_DC):
                nc.tensor.matmul(
                    ps,
                    lhsT=xt_sb[:, dc, 128 * lc : 128 * (lc + 1)],
                    rhs=wv_sb[:, dc, :],
                    start=(dc == 0),
                    stop=(dc == N_DC - 1),
                )
            for h in range(HPC):
                nc.vector.tensor_copy(
                    v_sb[:, lc, (H + 1) * h : (H + 1) * h + H],
                    ps[:, H * h : H * (h + 1)],
                )

        # ---- attention (scores transposed; 2-key-block groups) ----
        # Query-tile-outer so the partial output projection for tile t
        # overlaps the attention of tile t+1.  attn_all chunk j holds
        # heads (2j, 2j+1) on partition halves, matching the wp_sb row
        # layout for the final contraction.
        attn_all = singles.tile([128, 2, L], bf)
        for t in range(N_QT):
            qsl = slice(QT * t, QT * (t + 1))
            for h in range(HPC):
                qc = h // 2
                kc = 2 + h // 2
                base = 64 * (h % 2)
                q_all = qk_roped[base : base + 64, qc, :]
                k_all = qk_roped[base : base + 64, kc, :]
                po = ps_pv.tile([H + 1, QT], f32, tag="pv")
                n_kb = 4 * (t + 1)
                for g in range(n_kb // 2):
                    pss = ps_scores.tile([128, 2 * QT], f32, tag="scores")
                    et = epool.tile([128, 2 * QT], bf, tag="etile")
                    for j in range(2):
                        kb = 2 * g + j
                        d = 128 * kb - QT * t  # kb/qt diagonal offset
                        lo = max(d, 0)
                        nc.tensor.matmul(
                            pss[:, QT * j + lo : QT * (j + 1)],
                            lhsT=k_all[:, 128 * kb : 128 * (kb + 1)],
                            rhs=q_all[:, QT * t + lo : QT * (t + 1)],
                            start=True,
                            stop=True,
                        )
                    # exp (with 1/sqrt(H) scale); diag blocks get separate
                    # calls restricted to their valid column range
                    if 128 * (2 * g + 1) - QT * t < 0:
                        nc.scalar.activation(
                            out=et, in_=pss, func=Exp, scale=SCALE
                        )
                    else:
                        for j in range(2):
                            kb = 2 * g + j
                            lo = max(128 * kb - QT * t, 0)
                            nc.scalar.activation(
                                out=et[:, QT * j + lo : QT * (j + 1)],
                                in_=pss[:, QT * j + lo : QT * (j + 1)],
                                func=Exp,
                                scale=SCALE,
                            )
                    for j in range(2):
                        kb = 2 * g + j
                        d = 128 * kb - QT * t
                        lo = max(d, 0)
                        if d >= -127:
                            # boundary block: zero strictly-masked entries
                            nc.vector.tensor_mul(
                                et[:, QT * j + lo : QT * j + lo + 128],
                                et[:, QT * j + lo : QT * j + lo + 128],
                                tri_sb,
                            )
                        nc.tensor.matmul(
                            po[:, lo:QT],
                            lhsT=v_sb[:, kb, (H + 1) * h : (H + 1) * (h + 1)],
                            rhs=et[:, QT * j + lo : QT * (j + 1)],
                            start=(kb == 0),
                            stop=(kb == n_kb - 1),
                        )
                # normalize: attn = po[0:64] * (1 / po[64]).  The 1/z row
                # broadcasts across 64 partitions via a DVE partition-base
                # shift (64 -> 0) + gpsimd partition_broadcast; the odd
                # head's result lands on partitions 64..127 via a DVE
                # output-side partition shift.  No DRAM round-trips.
                z0 = dpool.tile([1, QT], f32, tag="z0")
                nc.vector.reciprocal(out=z0, in_=po[H : H + 1, :])
                rb = dpool.tile([H, QT], f32, tag="rb")
                nc.gpsimd.partition_broadcast(rb, z0)
                nc.vector.tensor_mul(
                    attn_all[base : base + H, h // 2, qsl], po[0:H, :], rb
                )

            # ---- partial output projection for this tile's 4 L-chunks
            # (contraction over this core's 4 heads = 2 chunks of 128) ----
            for lc in range(4 * t, 4 * (t + 1)):
                lsl = slice(128 * lc, 128 * (lc + 1))
                osb = opool.tile([128, D], bf, tag="osb")
                for oc in range(2):
                    osl = slice(QT * oc, QT * (oc + 1))
                    ps = ps_proj.tile([128, QT], f32, tag="proj")
                    for j in range(2):
                        nc.tensor.matmul(
                            ps,
                            lhsT=attn_all[:, j, lsl],
                            rhs=wp_sb[:, j, osl],
                            start=(j == 0),
                            stop=(j == 1),
                        )
                    nc.vector.tensor_copy(osb[:, osl], ps)
                if lc % 2 == 0:
                    nc.sync.dma_start(out=out[lsl, :], in_=osb)
                else:
                    nc.gpsimd.dma_start(out=out[lsl, :], in_=osb)

    nc.compile()
    _prog_cache["nc"] = nc
    return nc


def _host_inputs(x, rope, w_qkv, w_proj):
    """Shard + reformat the full inputs for the 8 cores."""
    rope = np.asarray(rope, dtype=np.float32)
    x = np.asarray(x, dtype=np.float32)
    w_qkv = np.asarray(w_qkv, dtype=np.float32)
    w_proj = np.asarray(w_proj, dtype=np.float32)

    xt_b = [np.ascontiguousarray(x[b].T).astype(BF16) for b in range(B)]

    # rope tables in h-major chunk layout: partition p of a 2-head chunk is
    # head (p // 64), component (p % 64); pair index i = (p % 64) // 2
    i_of_p = (np.arange(128) % 64) // 2
    cos_li = rope[:, :, 0]  # (L, 32)
    sin_li = rope[:, :, 1]
    ctab = np.ascontiguousarray(cos_li[:, i_of_p].T).astype(BF16)
    sign = np.where(np.arange(128) % 2 == 0, -1.0, 1.0).astype(np.float32)
    stab = np.ascontiguousarray((sin_li[:, i_of_p] * sign[None, :]).T).astype(BF16)

    # tri[p, f] = 1 where key offset p <= query offset f (keep), else 0
    tri = (np.arange(128)[:, None] <= np.arange(128)[None, :]).astype(BF16)

    in_maps = []
    for c in range(NCORES):
        b, g = divmod(c, GROUP)
        heads = [HPC * g + i for i in range(HPC)]
        wq = np.concatenate([w_qkv[:, H * n : H * (n + 1)] for n in heads], 1)
        wk = np.concatenate(
            [w_qkv[:, D + H * n : D + H * (n + 1)] for n in heads], 1
        )
        wvv = np.concatenate(
            [w_qkv[:, 2 * D + H * n : 2 * D + H * (n + 1)] for n in heads], 1
        )
        in_maps.append(
            {
                "xt": xt_b[b],
                "wqk": np.ascontiguousarray(
                    np.concatenate([wq, wk], 1)
                ).astype(BF16),
                "wv": np.ascontiguousarray(wvv).astype(BF16),
                "wp": np.ascontiguousarray(
                    w_proj[WPR * g : WPR * (g + 1), :]
                ).astype(BF16),
                "ctab": ctab,
                "stab": stab,
                "tri": tri,
            }
        )
    return in_maps


def kernel(x, rope, mask, w_qkv, w_proj, _trace=False):
    from concourse.bass_utils import run_bass_kernel_spmd

    nc = _build_program()
    in_maps = _host_inputs(x, rope, w_qkv, w_proj)
    res = run_bass_kernel_spmd(
        nc, in_maps, core_ids=list(range(NCORES)), trace=_trace
    )
    _prog_cache["last_result"] = res

    full = np.empty((B, L, D), dtype=np.float32)
    for b in range(B):
        acc = np.zeros((L, D), dtype=np.float32)
        for g in range(GROUP):
            acc += np.asarray(res.results[GROUP * b + g]["out"], dtype=np.float32)
        full[b] = acc
    return full


# revision 23
# speedup vs baseline: 1.2870x; 1.0315x over previous
"""Trainium2 Bass kernel for causal self-attention with RoPE.

Problem shapes (hardcoded): B=2, L=2048, D=1024, N=16 heads, H=64.

Sharding (8 cores, fully collective-free): data-parallel over batch
(2 groups of 4 cores), tensor-parallel over heads within a group
(4 heads/core).  Each core:
  1. computes q,k for its 4 heads in h-major layout (transposed matmul
     orientation: lhsT = w columns, rhs = x^T), applies RoPE on-chip,
  2. computes v in L-major layout (normal orientation),
  3. runs causal flash-style attention with scores transposed
     (S^T[key, query]) so softmax sums ride a fused ones-column through
     the PV matmul (no transposes anywhere),
  4. computes its PARTIAL output projection: its 4 heads' attention
     outputs (256 contraction dims) times the matching 256-row slice of
     w_proj, giving a full [L, D] partial in bf16.
Host code reformats/shards inputs (transpose, bf16 cast, column
permutation, table replication) and sums the 4 partials per batch.
There is no cross-core communication anywhere, so each core's
execution span is independent of peer launch skew.
"""

import numpy as np
import ml_dtypes

B, L, D, N_HEADS, H = 2, 2048, 1024, 16, 64
HPC = 4          # heads per core
GROUP = 4        # cores per batch group
NCORES = 8
QT = 512         # query tile width (matmul free dim)
KB = 128         # key block (psum partition dim)
N_QT = L // QT   # 4 query tiles
N_DC = D // 128  # 8 contraction chunks
N_LC = L // 128  # 16 L chunks for v / output rows
WPR = HPC * H    # w_proj rows per core (256)
BF16 = ml_dtypes.bfloat16

_prog_cache = {}


def _build_program():
    if "nc" in _prog_cache:
        return _prog_cache["nc"]

    import concourse.bass as bass
    import concourse.mybir as mybir
    import concourse.tile as tile
    from concourse import bacc
    from contextlib import ExitStack

    bf = mybir.dt.bfloat16
    f32 = mybir.dt.float32

    nc = bacc.Bacc(num_devices=NCORES)

    # host pre-packs everything into the SBUF layout: [128, ...free dims]
    xt = nc.dram_tensor("xt", [128, N_QT * N_DC * QT], bf, kind="ExternalInput")
    wqk = nc.dram_tensor("wqk", [128, N_DC * 4 * 128], bf, kind="ExternalInput")
    wv = nc.dram_tensor("wv", [128, N_DC * HPC * H], bf, kind="ExternalInput")
    wp = nc.dram_tensor("wp", [128, 2 * D], bf, kind="ExternalInput")
    ctab = nc.dram_tensor("ctab", [128, L], bf, kind="ExternalInput")
    stab = nc.dram_tensor("stab", [128, L], bf, kind="ExternalInput")
    tri = nc.dram_tensor("tri", [128, 128], bf, kind="ExternalInput")
    out = nc.dram_tensor("out", [L, D], bf, kind="ExternalOutput")

    Exp = mybir.ActivationFunctionType.Exp
    Copy = mybir.ActivationFunctionType.Copy
    SCALE = 1.0 / 8.0  # 1/sqrt(H)

    with tile.TileContext(nc) as tc, ExitStack() as ctx:
        singles = ctx.enter_context(tc.tile_pool(name="singles", bufs=1))
        work = ctx.enter_context(tc.tile_pool(name="work", bufs=4))
        epool = ctx.enter_context(tc.tile_pool(name="epool", bufs=4))
        dpool = ctx.enter_context(tc.tile_pool(name="dpool", bufs=3))
        opool = ctx.enter_context(tc.tile_pool(name="opool", bufs=3))
        ps_scores = ctx.enter_context(
            tc.tile_pool(name="ps_scores", bufs=2, space="PSUM")
        )
        ps_pv = ctx.enter_context(tc.tile_pool(name="ps_pv", bufs=2, space="PSUM"))
        ps_proj = ctx.enter_context(
            tc.tile_pool(name="ps_proj", bufs=2, space="PSUM")
        )

        # ---- load inputs to SBUF.  Host pre-packs every tensor into its
        # SBUF-resident layout so each load is ONE dma_start with big
        # contiguous per-partition runs: xt arrives as 4 query-tile slabs
        # [128, dc, 512] so the first q,k matmul chain starts after ~1 MB
        # lands; weights/tables are single loads on the pool queue. ----
        xt_sb = singles.tile([128, N_QT, N_DC, QT], bf)
        wqk_sb = singles.tile([128, N_DC, 4, 128], bf)
        wv_sb = singles.tile([128, N_DC, HPC * H], bf)
        wp_sb = singles.tile([128, 2, D], bf)
        ctab_sb = singles.tile([128, L], bf)
        stab_sb = singles.tile([128, L], bf)
        tri_sb = singles.tile([128, 128], bf)
        SLAB = N_DC * QT
        # first slab + wqk split into dc-halves so the first q,k chain
        # starts after ~0.5 MB instead of 2 MB (DMA engines serialize)
        HALF = SLAB // 2
        nc.gpsimd.dma_start(
            out=wv_sb[:, :, :],
            in_=wv[:, :].rearrange("p (dc m) -> p dc m", dc=N_DC),
        )
        for half in range(2):
            nc.sync.dma_start(
                out=xt_sb[:, 0, 4 * half : 4 * (half + 1), :],
                in_=xt[:, HALF * half : HALF * (half + 1)].rearrange(
                    "p (dc c) -> p dc c", dc=4
                ),
            )
            nc.gpsimd.dma_start(
                out=wqk_sb[:, 4 * half : 4 * (half + 1), :, :],
                in_=wqk[
                    :, 4 * 4 * 128 * half : 4 * 4 * 128 * (half + 1)
                ].rearrange("p (dc qc m) -> p dc qc m", dc=4, qc=4),
            )
        for lt in range(1, N_QT):
            nc.sync.dma_start(
                out=xt_sb[:, lt, :, :],
                in_=xt[:, SLAB * lt : SLAB * (lt + 1)].rearrange(
                    "p (dc c) -> p dc c", dc=N_DC
                ),
            )
        nc.gpsimd.dma_start(out=ctab_sb, in_=ctab[:, :])
        nc.gpsimd.dma_start(out=stab_sb, in_=stab[:, :])
        nc.gpsimd.dma_start(out=tri_sb, in_=tri[:, :])
        # w_proj rows for this core's 4 heads: chunk j holds rows for
        # heads (2j, 2j+1) stacked across the 128 partitions
        nc.gpsimd.dma_start(
            out=wp_sb[:, :, :],
            in_=wp[:, :].rearrange("p (j m) -> p j m", j=2),
        )

        # ---- v projection (normal orientation), with ones column fused ----
        # per L-chunk layout: [v_h0(64) 1 | v_h1(64) 1 | v_h2(64) 1 | v_h3(64) 1]
        v_sb = singles.tile([128, N_LC, HPC * (H + 1)], bf)
        for h in range(HPC):
            nc.vector.memset(v_sb[:, :, (H + 1) * h + H], 1.0)
        for lc in range(N_LC):
            ps = ps_proj.tile([128, HPC * H], f32, tag="proj")
            for dc in range(N_DC):
                nc.tensor.matmul(
                    ps,
                    lhsT=xt_sb[:, lc // 4, dc, 128 * (lc % 4) : 128 * (lc % 4 + 1)],
                    rhs=wv_sb[:, dc, :],
                    start=(dc == 0),
                    stop=(dc == N_DC - 1),
                )
            for h in range(HPC):
                nc.vector.tensor_copy(
                    v_sb[:, lc, (H + 1) * h : (H + 1) * h + H],
                    ps[:, H * h : H * (h + 1)],
                )

        # ---- q,k projection (transposed orientation) + RoPE ----
        # lt-outer so the first chains only need xt slab 0; one long PE
        # phase keeps the tensor engine at full p-state.
        # qk chunks: 0,1 = q heads (0,1),(2,3); 2,3 = k heads (0,1),(2,3)
        qk_roped = singles.tile([128, 4, L], bf)
        for lt in range(N_QT):
            lsl = slice(QT * lt, QT * (lt + 1))
            for qc in range(4):
                ps = ps_proj.tile([128, QT], f32, tag="proj")
                for dc in range(N_DC):
                    nc.tensor.matmul(
                        ps,
                        lhsT=wqk_sb[:, dc, qc, :],
                        rhs=xt_sb[:, lt, dc, :],
                        start=(dc == 0),
                        stop=(dc == N_DC - 1),
                    )
                qk_bf = work.tile([128, QT], bf, tag="qkbf")
                nc.scalar.activation(out=qk_bf, in_=ps, func=Copy)
                # rot[p] = qk_bf[p ^ 1]  (adjacent even/odd partner swap,
                # a within-32-partition permutation -> stream_shuffle)
                rot = work.tile([128, QT], bf, tag="rot")
                nc.vector.stream_shuffle(
                    rot, qk_bf, mask=[i ^ 1 for i in range(32)]
                )
                m1 = work.tile([128, QT], bf, tag="m1")
                nc.vector.tensor_mul(m1, qk_bf, ctab_sb[:, lsl])
                m2 = work.tile([128, QT], bf, tag="m2")
                nc.vector.tensor_mul(m2, rot, stab_sb[:, lsl])
                nc.vector.tensor_add(qk_roped[:, qc, lsl], m1, m2)

        # ---- attention (scores transposed; 2-key-block groups) ----
        # Query-tile-outer so the partial output projection for tile t
        # overlaps the attention of tile t+1.  attn_all chunk j holds
        # heads (2j, 2j+1) on partition halves, matching the wp_sb row
        # layout for the final contraction.
        attn_all = singles.tile([128, 2, L], bf)

        def _out_proj_tile(tp):
            # partial output projection for tile tp's 4 L-chunks
            # (contraction over this core's 4 heads = 2 chunks of 128)
            for lc in range(4 * tp, 4 * (tp + 1)):
                lsl = slice(128 * lc, 128 * (lc + 1))
                osb = opool.tile([128, D], bf, tag="osb", name="osb")
                for oc in range(2):
                    osl = slice(QT * oc, QT * (oc + 1))
                    ps = ps_proj.tile([128, QT], f32, tag="proj", name="ps")
                    for j in range(2):
                        nc.tensor.matmul(
                            ps,
                            lhsT=attn_all[:, j, lsl],
                            rhs=wp_sb[:, j, osl],
                            start=(j == 0),
                            stop=(j == 1),
                        )
                    nc.vector.tensor_copy(osb[:, osl], ps)
                if lc % 2 == 0:
                    nc.sync.dma_start(out=out[lsl, :], in_=osb)
                else:
                    nc.gpsimd.dma_start(out=out[lsl, :], in_=osb)

        for t in range(N_QT):
            qsl = slice(QT * t, QT * (t + 1))
            for h in range(HPC):
                qc = h // 2
                kc = 2 + h // 2
                base = 64 * (h % 2)
                q_all = qk_roped[base : base + 64, qc, :]
                k_all = qk_roped[base : base + 64, kc, :]
                po = ps_pv.tile([H + 1, QT], f32, tag="pv")
                n_kb = 4 * (t + 1)
                for g in range(n_kb // 2):
                    pss = ps_scores.tile([128, 2 * QT], f32, tag="scores")
                    et = epool.tile([128, 2 * QT], bf, tag="etile")
                    for j in range(2):
                        kb = 2 * g + j
                        d = 128 * kb - QT * t  # kb/qt diagonal offset
                        lo = max(d, 0)
                        nc.tensor.matmul(
                            pss[:, QT * j + lo : QT * (j + 1)],
                            lhsT=k_all[:, 128 * kb : 128 * (kb + 1)],
                            rhs=q_all[:, QT * t + lo : QT * (t + 1)],
                            start=True,
                            stop=True,
                        )
                    # exp (with 1/sqrt(H) scale); diag blocks get separate
                    # calls restricted to their valid column range
                    if 128 * (2 * g + 1) - QT * t < 0:
                        nc.scalar.activation(
                            out=et, in_=pss, func=Exp, scale=SCALE
                        )
                    else:
                        for j in range(2):
                            kb = 2 * g + j
                            lo = max(128 * kb - QT * t, 0)
                            nc.scalar.activation(
                                out=et[:, QT * j + lo : QT * (j + 1)],
                                in_=pss[:, QT * j + lo : QT * (j + 1)],
                                func=Exp,
                                scale=SCALE,
                            )
                    for j in range(2):
                        kb = 2 * g + j
                        d = 128 * kb - QT * t
                        lo = max(d, 0)
                        if d >= -127:
                            # boundary block: zero strictly-masked entries
                            nc.vector.tensor_mul(
                                et[:, QT * j + lo : QT * j + lo + 128],
                                et[:, QT * j + lo : QT * j + lo + 128],
                                tri_sb,
                            )
                        nc.tensor.matmul(
                            po[:, lo:QT],
                            lhsT=v_sb[:, kb, (H + 1) * h : (H + 1) * (h + 1)],
                            rhs=et[:, QT * j + lo : QT * (j + 1)],
                            start=(kb == 0),
                            stop=(kb == n_kb - 1),
                        )
                # normalize: attn = po[0:64] * (1 / po[64]).  The 1/z row
                # broadcasts across 64 partitions via a DVE partition-base
                # shift (64 -> 0) + gpsimd partition_broadcast; the odd
                # head's result lands on partitions 64..127 via a DVE
                # output-side partition shift.  No DRAM round-trips.
                z0 = dpool.tile([1, QT], f32, tag="z0")
                nc.vector.reciprocal(out=z0, in_=po[H : H + 1, :])
                rb = dpool.tile([H, QT], f32, tag="rb")
                nc.gpsimd.partition_broadcast(rb, z0)
                nc.vector.tensor_mul(
                    attn_all[base : base + H, h // 2, qsl], po[0:H, :], rb
                )
                if h == 0 and t > 0:
                    # ---- partial output projection for the PREVIOUS
                    # tile, emitted after this tile's first head so the
                    # PE never waits on the previous tile's last
                    # normalize chain (DVE reciprocal -> pool broadcast
                    # -> DVE mul latency is hidden behind scores) ----
                    _out_proj_tile(t - 1)

        _out_proj_tile(N_QT - 1)

    nc.compile()
    _prog_cache["nc"] = nc
    return nc


def _host_inputs(x, rope, w_qkv, w_proj):
    """Shard + reformat the full inputs for the 8 cores."""
    rope = np.asarray(rope, dtype=np.float32)
    x = np.asarray(x, dtype=np.float32)
    w_qkv = np.asarray(w_qkv, dtype=np.float32)
    w_proj = np.asarray(w_proj, dtype=np.float32)

    # xt packed as [128, lt, dc, c]: xt[p, lt, dc, c] = x[b][lt*512+c, dc*128+p]
    xt_b = []
    for b in range(B):
        xb = x[b].T.reshape(N_DC, 128, N_QT, QT)  # [dc, p, lt, c]
        xt_b.append(
            np.ascontiguousarray(xb.transpose(1, 2, 0, 3))
            .reshape(128, N_QT * N_DC * QT)
            .astype(BF16)
        )

    # rope tables in h-major chunk layout: partition p of a 2-head chunk is
    # head (p // 64), component (p % 64); pair index i = (p % 64) // 2
    i_of_p = (np.arange(128) % 64) // 2
    cos_li = rope[:, :, 0]  # (L, 32)
    sin_li = rope[:, :, 1]
    ctab = np.ascontiguousarray(cos_li[:, i_of_p].T).astype(BF16)
    sign = np.where(np.arange(128) % 2 == 0, -1.0, 1.0).astype(np.float32)
    stab = np.ascontiguousarray((sin_li[:, i_of_p] * sign[None, :]).T).astype(BF16)

    # tri[p, f] = 1 where key offset p <= query offset f (keep), else 0
    tri = (np.arange(128)[:, None] <= np.arange(128)[None, :]).astype(BF16)

    in_maps = []
    for c in range(NCORES):
        b, g = divmod(c, GROUP)
        heads = [HPC * g + i for i in range(HPC)]
        wq = np.concatenate([w_qkv[:, H * n : H * (n + 1)] for n in heads], 1)
        wk = np.concatenate(
            [w_qkv[:, D + H * n : D + H * (n + 1)] for n in heads], 1
        )
        wvv = np.concatenate(
            [w_qkv[:, 2 * D + H * n : 2 * D + H * (n + 1)] for n in heads], 1
        )
        # wqk packed as [128, dc, qc, m]; wv as [128, dc, m]; wp as [128, j, m]
        wqk_c = np.concatenate([wq, wk], 1)  # [1024, 1024]
        wqk_p = (
            wqk_c.reshape(N_DC, 128, 4, 128)
            .transpose(1, 0, 2, 3)
            .reshape(128, N_DC * 4 * 128)
        )
        wv_p = (
            wvv.reshape(N_DC, 128, HPC * H)
            .transpose(1, 0, 2)
            .reshape(128, N_DC * HPC * H)
        )
        wp_p = (
            w_proj[WPR * g : WPR * (g + 1), :]
            .reshape(2, 128, D)
            .transpose(1, 0, 2)
            .reshape(128, 2 * D)
        )
        in_maps.append(
            {
                "xt": xt_b[b],
                "wqk": np.ascontiguousarray(wqk_p).astype(BF16),
                "wv": np.ascontiguousarray(wv_p).astype(BF16),
                "wp": np.ascontiguousarray(wp_p).astype(BF16),
                "ctab": ctab,
                "stab": stab,
                "tri": tri,
            }
        )
    return in_maps


def kernel(x, rope, mask, w_qkv, w_proj, _trace=False):
    from concourse.bass_utils import run_bass_kernel_spmd

    nc = _build_program()
    in_maps = _host_inputs(x, rope, w_qkv, w_proj)
    res = run_bass_kernel_spmd(
        nc, in_maps, core_ids=list(range(NCORES)), trace=_trace
    )
    _prog_cache["last_result"] = res

    full = np.empty((B, L, D), dtype=np.float32)
    for b in range(B):
        acc = np.zeros((L, D), dtype=np.float32)
        for g in range(GROUP):
            acc += np.asarray(res.results[GROUP * b + g]["out"], dtype=np.float32)
        full[b] = acc
    return full


# revision 34
# speedup vs baseline: 1.2992x; 1.0095x over previous
"""Trainium2 Bass kernel for causal self-attention with RoPE.

Problem shapes (hardcoded): B=2, L=2048, D=1024, N=16 heads, H=64.

Sharding (8 cores, fully collective-free): data-parallel over batch
(2 groups of 4 cores), tensor-parallel over heads within a group
(4 heads/core).  Each core:
  1. computes q,k for its 4 heads in h-major layout (transposed matmul
     orientation: lhsT = w columns, rhs = x^T), applies RoPE on-chip,
  2. computes v in L-major layout (normal orientation),
  3. runs causal flash-style attention with scores transposed
     (S^T[key, query]) so softmax sums ride a fused ones-column through
     the PV matmul (no transposes anywhere),
  4. computes its PARTIAL output projection: its 4 heads' attention
     outputs (256 contraction dims) times the matching 256-row slice of
     w_proj, giving a full [L, D] partial in bf16.
Host code reformats/shards inputs (transpose, bf16 cast, column
permutation, table replication) and sums the 4 partials per batch.
There is no cross-core communication anywhere, so each core's
execution span is independent of peer launch skew.
"""

import numpy as np
import ml_dtypes

B, L, D, N_HEADS, H = 2, 2048, 1024, 16, 64
HPC = 4          # heads per core
GROUP = 4        # cores per batch group
NCORES = 8
QT = 512         # query tile width (matmul free dim)
KB = 128         # key block (psum partition dim)
N_QT = L // QT   # 4 query tiles
N_DC = D // 128  # 8 contraction chunks
N_LC = L // 128  # 16 L chunks for v / output rows
WPR = HPC * H    # w_proj rows per core (256)
BF16 = ml_dtypes.bfloat16

_prog_cache = {}


def _build_program():
    if "nc" in _prog_cache:
        return _prog_cache["nc"]

    import concourse.bass as bass
    import concourse.mybir as mybir
    import concourse.tile as tile
    from concourse import bacc
    from contextlib import ExitStack

    bf = mybir.dt.bfloat16
    f32 = mybir.dt.float32

    nc = bacc.Bacc(num_devices=NCORES)

    # host pre-packs everything into the SBUF layout: [128, ...free dims]
    xt = nc.dram_tensor("xt", [128, N_QT * N_DC * QT], bf, kind="ExternalInput")
    wqk = nc.dram_tensor("wqk", [128, N_DC * 4 * 128], bf, kind="ExternalInput")
    wv = nc.dram_tensor("wv", [128, N_DC * HPC * H], bf, kind="ExternalInput")
    wp = nc.dram_tensor("wp", [128, 2 * D], bf, kind="ExternalInput")
    ctab = nc.dram_tensor("ctab", [128, L], bf, kind="ExternalInput")
    stab = nc.dram_tensor("stab", [128, L], bf, kind="ExternalInput")
    tri = nc.dram_tensor("tri", [128, 128], bf, kind="ExternalInput")
    out = nc.dram_tensor("out", [L, D], bf, kind="ExternalOutput")

    Exp = mybir.ActivationFunctionType.Exp
    Copy = mybir.ActivationFunctionType.Copy
    SCALE = 1.0 / 8.0  # 1/sqrt(H)

    with tile.TileContext(nc) as tc, ExitStack() as ctx:
        singles = ctx.enter_context(tc.tile_pool(name="singles", bufs=1))
        work = ctx.enter_context(tc.tile_pool(name="work", bufs=6))
        epool = ctx.enter_context(tc.tile_pool(name="epool", bufs=6))
        dpool = ctx.enter_context(tc.tile_pool(name="dpool", bufs=4))
        opool = ctx.enter_context(tc.tile_pool(name="opool", bufs=4))
        ps_scores = ctx.enter_context(
            tc.tile_pool(name="ps_scores", bufs=2, space="PSUM")
        )
        ps_pv = ctx.enter_context(tc.tile_pool(name="ps_pv", bufs=2, space="PSUM"))
        ps_proj = ctx.enter_context(
            tc.tile_pool(name="ps_proj", bufs=2, space="PSUM")
        )

        # ---- load inputs to SBUF.  Host pre-packs every tensor into its
        # SBUF-resident layout so each load is ONE dma_start with big
        # contiguous per-partition runs: xt arrives as 4 query-tile slabs
        # [128, dc, 512] so the first q,k matmul chain starts after ~1 MB
        # lands; weights/tables are single loads on the pool queue. ----
        xt_sb = singles.tile([128, N_QT, N_DC, QT], bf)
        wqk_sb = singles.tile([128, N_DC, 4, 128], bf)
        wv_sb = singles.tile([128, N_DC, HPC * H], bf)
        wp_sb = singles.tile([128, 2, D], bf)
        ctab_sb = singles.tile([128, L], bf)
        stab_sb = singles.tile([128, L], bf)
        tri_sb = singles.tile([128, 128], bf)
        SLAB = N_DC * QT
        # first slab + wqk split into dc-halves so the first q,k chain
        # starts after ~0.5 MB instead of 2 MB (DMA engines serialize)
        HALF = SLAB // 2
        nc.gpsimd.dma_start(
            out=wv_sb[:, :, :],
            in_=wv[:, :].rearrange("p (dc m) -> p dc m", dc=N_DC),
        )
        for half in range(2):
            nc.sync.dma_start(
                out=xt_sb[:, 0, 4 * half : 4 * (half + 1), :],
                in_=xt[:, HALF * half : HALF * (half + 1)].rearrange(
                    "p (dc c) -> p dc c", dc=4
                ),
            )
            nc.gpsimd.dma_start(
                out=wqk_sb[:, 4 * half : 4 * (half + 1), :, :],
                in_=wqk[
                    :, 4 * 4 * 128 * half : 4 * 4 * 128 * (half + 1)
                ].rearrange("p (dc qc m) -> p dc qc m", dc=4, qc=4),
            )
        for lt in range(1, N_QT):
            nc.sync.dma_start(
                out=xt_sb[:, lt, :, :],
                in_=xt[:, SLAB * lt : SLAB * (lt + 1)].rearrange(
                    "p (dc c) -> p dc c", dc=N_DC
                ),
            )
        nc.gpsimd.dma_start(out=ctab_sb, in_=ctab[:, :])
        nc.gpsimd.dma_start(out=stab_sb, in_=stab[:, :])
        nc.gpsimd.dma_start(out=tri_sb, in_=tri[:, :])
        # w_proj rows for this core's 4 heads: chunk j holds rows for
        # heads (2j, 2j+1) stacked across the 128 partitions
        nc.gpsimd.dma_start(
            out=wp_sb[:, :, :],
            in_=wp[:, :].rearrange("p (j m) -> p j m", j=2),
        )

        # ---- v projection (normal orientation), with ones column fused ----
        # per L-chunk layout: [v_h0(64) 1 | v_h1(64) 1 | v_h2(64) 1 | v_h3(64) 1]
        v_sb = singles.tile([128, N_LC, HPC * (H + 1)], bf)
        for h in range(HPC):
            nc.vector.memset(v_sb[:, :, (H + 1) * h + H], 1.0)
        for lc in range(N_LC):
            ps = ps_proj.tile([128, HPC * H], f32, tag="proj")
            for dc in range(N_DC):
                nc.tensor.matmul(
                    ps,
                    lhsT=xt_sb[:, lc // 4, dc, 128 * (lc % 4) : 128 * (lc % 4 + 1)],
                    rhs=wv_sb[:, dc, :],
                    start=(dc == 0),
                    stop=(dc == N_DC - 1),
                )
            for h in range(HPC):
                nc.vector.tensor_copy(
                    v_sb[:, lc, (H + 1) * h : (H + 1) * h + H],
                    ps[:, H * h : H * (h + 1)],
                )

        # ---- q,k projection (transposed orientation) + RoPE ----
        # lt-outer so the first chains only need xt slab 0; one long PE
        # phase keeps the tensor engine at full p-state.
        # qk chunks: 0,1 = q heads (0,1),(2,3); 2,3 = k heads (0,1),(2,3)
        qk_roped = singles.tile([128, 4, L], bf)
        for lt in range(N_QT):
            lsl = slice(QT * lt, QT * (lt + 1))
            for qc in range(4):
                ps = ps_proj.tile([128, QT], f32, tag="proj")
                for dc in range(N_DC):
                    nc.tensor.matmul(
                        ps,
                        lhsT=wqk_sb[:, dc, qc, :],
                        rhs=xt_sb[:, lt, dc, :],
                        start=(dc == 0),
                        stop=(dc == N_DC - 1),
                    )
                qk_bf = work.tile([128, QT], bf, tag="qkbf")
                nc.scalar.activation(out=qk_bf, in_=ps, func=Copy)
                # rot[p] = qk_bf[p ^ 1]  (adjacent even/odd partner swap,
                # a within-32-partition permutation -> stream_shuffle)
                rot = work.tile([128, QT], bf, tag="rot")
                nc.vector.stream_shuffle(
                    rot, qk_bf, mask=[i ^ 1 for i in range(32)]
                )
                m1 = work.tile([128, QT], bf, tag="m1")
                nc.vector.tensor_mul(m1, qk_bf, ctab_sb[:, lsl])
                m2 = work.tile([128, QT], bf, tag="m2")
                nc.vector.tensor_mul(m2, rot, stab_sb[:, lsl])
                nc.gpsimd.tensor_add(qk_roped[:, qc, lsl], m1, m2)

        # ---- attention (scores transposed; 2-key-block groups) ----
        # Query-tile-outer so the partial output projection for tile t
        # overlaps the attention of tile t+1.  attn_all chunk j holds
        # heads (2j, 2j+1) on partition halves, matching the wp_sb row
        # layout for the final contraction.
        attn_all = singles.tile([128, 2, L], bf)

        def _out_proj_tile(tp):
            # partial output projection for tile tp's 4 L-chunks
            # (contraction over this core's 4 heads = 2 chunks of 128)
            for lc in range(4 * tp, 4 * (tp + 1)):
                lsl = slice(128 * lc, 128 * (lc + 1))
                osb = opool.tile([128, D], bf, tag="osb", name="osb")
                for oc in range(2):
                    osl = slice(QT * oc, QT * (oc + 1))
                    ps = ps_proj.tile([128, QT], f32, tag="proj", name="ps")
                    for j in range(2):
                        nc.tensor.matmul(
                            ps,
                            lhsT=attn_all[:, j, lsl],
                            rhs=wp_sb[:, j, osl],
                            start=(j == 0),
                            stop=(j == 1),
                        )
                    nc.vector.tensor_copy(osb[:, osl], ps)
                if lc % 2 == 0:
                    nc.sync.dma_start(out=out[lsl, :], in_=osb)
                else:
                    nc.gpsimd.dma_start(out=out[lsl, :], in_=osb)

        for t in range(N_QT):
            qsl = slice(QT * t, QT * (t + 1))
            for h in range(HPC):
                qc = h // 2
                kc = 2 + h // 2
                base = 64 * (h % 2)
                q_all = qk_roped[base : base + 64, qc, :]
                k_all = qk_roped[base : base + 64, kc, :]
                po = ps_pv.tile([H + 1, QT], f32, tag="pv")
                n_kb = 4 * (t + 1)
                for g in range(n_kb // 2):
                    pss = ps_scores.tile([128, 2 * QT], f32, tag="scores")
                    et = epool.tile([128, 2 * QT], bf, tag="etile")
                    for j in range(2):
                        kb = 2 * g + j
                        d = 128 * kb - QT * t  # kb/qt diagonal offset
                        lo = max(d, 0)
                        nc.tensor.matmul(
                            pss[:, QT * j + lo : QT * (j + 1)],
                            lhsT=k_all[:, 128 * kb : 128 * (kb + 1)],
                            rhs=q_all[:, QT * t + lo : QT * (t + 1)],
                            start=True,
                            stop=True,
                        )
                    # exp (with 1/sqrt(H) scale); diag blocks get separate
                    # calls restricted to their valid column range
                    if 128 * (2 * g + 1) - QT * t < 0:
                        nc.scalar.activation(
                            out=et, in_=pss, func=Exp, scale=SCALE
                        )
                    else:
                        for j in range(2):
                            kb = 2 * g + j
                            lo = max(128 * kb - QT * t, 0)
                            nc.scalar.activation(
                                out=et[:, QT * j + lo : QT * (j + 1)],
                                in_=pss[:, QT * j + lo : QT * (j + 1)],
                                func=Exp,
                                scale=SCALE,
                            )
                    for j in range(2):
                        kb = 2 * g + j
                        d = 128 * kb - QT * t
                        lo = max(d, 0)
                        if d >= -127:
                            # boundary block: zero strictly-masked entries
                            nc.vector.tensor_mul(
                                et[:, QT * j + lo : QT * j + lo + 128],
                                et[:, QT * j + lo : QT * j + lo + 128],
                                tri_sb,
                            )
                        nc.tensor.matmul(
                            po[:, lo:QT],
                            lhsT=v_sb[:, kb, (H + 1) * h : (H + 1) * (h + 1)],
                            rhs=et[:, QT * j + lo : QT * (j + 1)],
                            start=(kb == 0),
                            stop=(kb == n_kb - 1),
                        )
                # normalize: attn = po[0:64] * (1 / po[64]).  The 1/z row
                # broadcasts across 64 partitions via a DVE partition-base
                # shift (64 -> 0) + gpsimd partition_broadcast; the odd
                # head's result lands on partitions 64..127 via a DVE
                # output-side partition shift.  No DRAM round-trips.
                z0 = dpool.tile([1, QT], f32, tag="z0")
                nc.vector.reciprocal(out=z0, in_=po[H : H + 1, :])
                rb = dpool.tile([H, QT], f32, tag="rb")
                nc.gpsimd.partition_broadcast(rb, z0)
                nc.vector.tensor_mul(
                    attn_all[base : base + H, h // 2, qsl], po[0:H, :], rb
                )
                if h == 0 and t > 0:
                    # ---- partial output projection for the PREVIOUS
                    # tile, emitted after this tile's first head so the
                    # PE never waits on the previous tile's last
                    # normalize chain (DVE reciprocal -> pool broadcast
                    # -> DVE mul latency is hidden behind scores) ----
                    _out_proj_tile(t - 1)

        _out_proj_tile(N_QT - 1)

    nc.compile()
    _prog_cache["nc"] = nc
    return nc


def _host_inputs(x, rope, w_qkv, w_proj):
    """Shard + reformat the full inputs for the 8 cores."""
    rope = np.asarray(rope, dtype=np.float32)
    x = np.asarray(x, dtype=np.float32)
    w_qkv = np.asarray(w_qkv, dtype=np.float32)
    w_proj = np.asarray(w_proj, dtype=np.float32)

    # xt packed as [128, lt, dc, c]: xt[p, lt, dc, c] = x[b][lt*512+c, dc*128+p]
    xt_b = []
    for b in range(B):
        xb = x[b].T.reshape(N_DC, 128, N_QT, QT)  # [dc, p, lt, c]
        xt_b.append(
            np.ascontiguousarray(xb.transpose(1, 2, 0, 3))
            .reshape(128, N_QT * N_DC * QT)
            .astype(BF16)
        )

    # rope tables in h-major chunk layout: partition p of a 2-head chunk is
    # head (p // 64), component (p % 64); pair index i = (p % 64) // 2
    i_of_p = (np.arange(128) % 64) // 2
    cos_li = rope[:, :, 0]  # (L, 32)
    sin_li = rope[:, :, 1]
    ctab = np.ascontiguousarray(cos_li[:, i_of_p].T).astype(BF16)
    sign = np.where(np.arange(128) % 2 == 0, -1.0, 1.0).astype(np.float32)
    stab = np.ascontiguousarray((sin_li[:, i_of_p] * sign[None, :]).T).astype(BF16)

    # tri[p, f] = 1 where key offset p <= query offset f (keep), else 0
    tri = (np.arange(128)[:, None] <= np.arange(128)[None, :]).astype(BF16)

    in_maps = []
    for c in range(NCORES):
        b, g = divmod(c, GROUP)
        heads = [HPC * g + i for i in range(HPC)]
        wq = np.concatenate([w_qkv[:, H * n : H * (n + 1)] for n in heads], 1)
        wk = np.concatenate(
            [w_qkv[:, D + H * n : D + H * (n + 1)] for n in heads], 1
        )
        wvv = np.concatenate(
            [w_qkv[:, 2 * D + H * n : 2 * D + H * (n + 1)] for n in heads], 1
        )
        # wqk packed as [128, dc, qc, m]; wv as [128, dc, m]; wp as [128, j, m]
        wqk_c = np.concatenate([wq, wk], 1)  # [1024, 1024]
        wqk_p = (
            wqk_c.reshape(N_DC, 128, 4, 128)
            .transpose(1, 0, 2, 3)
            .reshape(128, N_DC * 4 * 128)
        )
        wv_p = (
            wvv.reshape(N_DC, 128, HPC * H)
            .transpose(1, 0, 2)
            .reshape(128, N_DC * HPC * H)
        )
        wp_p = (
            w_proj[WPR * g : WPR * (g + 1), :]
            .reshape(2, 128, D)
            .transpose(1, 0, 2)
            .reshape(128, 2 * D)
        )
        in_maps.append(
            {
                "xt": xt_b[b],
                "wqk": np.ascontiguousarray(wqk_p).astype(BF16),
                "wv": np.ascontiguousarray(wv_p).astype(BF16),
                "wp": np.ascontiguousarray(wp_p).astype(BF16),
                "ctab": ctab,
                "stab": stab,
                "tri": tri,
            }
        )
    return in_maps


def kernel(x, rope, mask, w_qkv, w_proj, _trace=False):
    from concourse.bass_utils import run_bass_kernel_spmd

    nc = _build_program()
    in_maps = _host_inputs(x, rope, w_qkv, w_proj)
    res = run_bass_kernel_spmd(
        nc, in_maps, core_ids=list(range(NCORES)), trace=_trace
    )
    _prog_cache["last_result"] = res

    full = np.empty((B, L, D), dtype=np.float32)
    for b in range(B):
        acc = np.zeros((L, D), dtype=np.float32)
        for g in range(GROUP):
            acc += np.asarray(res.results[GROUP * b + g]["out"], dtype=np.float32)
        full[b] = acc
    return full


# revision 35
# speedup vs baseline: 1.3053x; 1.0047x over previous
"""Trainium2 Bass kernel for causal self-attention with RoPE.

Problem shapes (hardcoded): B=2, L=2048, D=1024, N=16 heads, H=64.

Sharding (8 cores, fully collective-free): data-parallel over batch
(2 groups of 4 cores), tensor-parallel over heads within a group
(4 heads/core).  Each core:
  1. computes q,k for its 4 heads in h-major layout (transposed matmul
     orientation: lhsT = w columns, rhs = x^T), applies RoPE on-chip,
  2. computes v in L-major layout (normal orientation),
  3. runs causal flash-style attention with scores transposed
     (S^T[key, query]) so softmax sums ride a fused ones-column through
     the PV matmul (no transposes anywhere),
  4. computes its PARTIAL output projection: its 4 heads' attention
     outputs (256 contraction dims) times the matching 256-row slice of
     w_proj, giving a full [L, D] partial in bf16.
Host code reformats/shards inputs (transpose, bf16 cast, column
permutation, table replication) and sums the 4 partials per batch.
There is no cross-core communication anywhere, so each core's
execution span is independent of peer launch skew.
"""

import numpy as np
import ml_dtypes

B, L, D, N_HEADS, H = 2, 2048, 1024, 16, 64
HPC = 4          # heads per core
GROUP = 4        # cores per batch group
NCORES = 8
QT = 512         # query tile width (matmul free dim)
KB = 128         # key block (psum partition dim)
N_QT = L // QT   # 4 query tiles
N_DC = D // 128  # 8 contraction chunks
N_LC = L // 128  # 16 L chunks for v / output rows
WPR = HPC * H    # w_proj rows per core (256)
BF16 = ml_dtypes.bfloat16

_prog_cache = {}


def _build_program():
    if "nc" in _prog_cache:
        return _prog_cache["nc"]

    import concourse.bass as bass
    import concourse.mybir as mybir
    import concourse.tile as tile
    from concourse import bacc
    from contextlib import ExitStack

    bf = mybir.dt.bfloat16
    f32 = mybir.dt.float32

    nc = bacc.Bacc(num_devices=NCORES)

    # host pre-packs everything into the SBUF layout: [128, ...free dims]
    xt = nc.dram_tensor("xt", [128, N_QT * N_DC * QT], bf, kind="ExternalInput")
    wqk = nc.dram_tensor("wqk", [128, N_DC * 4 * 128], bf, kind="ExternalInput")
    wv = nc.dram_tensor("wv", [128, N_DC * HPC * H], bf, kind="ExternalInput")
    wp = nc.dram_tensor("wp", [128, 2 * D], bf, kind="ExternalInput")
    ctab = nc.dram_tensor("ctab", [128, L], bf, kind="ExternalInput")
    stab = nc.dram_tensor("stab", [128, L], bf, kind="ExternalInput")
    tri = nc.dram_tensor("tri", [128, 128], bf, kind="ExternalInput")
    out = nc.dram_tensor("out", [L, D], bf, kind="ExternalOutput")

    Exp = mybir.ActivationFunctionType.Exp
    Copy = mybir.ActivationFunctionType.Copy
    SCALE = 1.0 / 8.0  # 1/sqrt(H)

    with tile.TileContext(nc) as tc, ExitStack() as ctx:
        singles = ctx.enter_context(tc.tile_pool(name="singles", bufs=1))
        work = ctx.enter_context(tc.tile_pool(name="work", bufs=6))
        epool = ctx.enter_context(tc.tile_pool(name="epool", bufs=6))
        dpool = ctx.enter_context(tc.tile_pool(name="dpool", bufs=4))
        opool = ctx.enter_context(tc.tile_pool(name="opool", bufs=4))
        ps_scores = ctx.enter_context(
            tc.tile_pool(name="ps_scores", bufs=2, space="PSUM")
        )
        ps_pv = ctx.enter_context(tc.tile_pool(name="ps_pv", bufs=2, space="PSUM"))
        ps_proj = ctx.enter_context(
            tc.tile_pool(name="ps_proj", bufs=2, space="PSUM")
        )

        # ---- load inputs to SBUF.  Host pre-packs every tensor into its
        # SBUF-resident layout so each load is ONE dma_start with big
        # contiguous per-partition runs: xt arrives as 4 query-tile slabs
        # [128, dc, 512] so the first q,k matmul chain starts after ~1 MB
        # lands; weights/tables are single loads on the pool queue. ----
        xt_sb = singles.tile([128, N_QT, N_DC, QT], bf)
        wqk_sb = singles.tile([128, N_DC, 4, 128], bf)
        wv_sb = singles.tile([128, N_DC, HPC * H], bf)
        wp_sb = singles.tile([128, 2, D], bf)
        ctab_sb = singles.tile([128, L], bf)
        stab_sb = singles.tile([128, L], bf)
        tri_sb = singles.tile([128, 128], bf)
        SLAB = N_DC * QT
        # first slab + wqk split into dc-halves so the first q,k chain
        # starts after ~0.5 MB instead of 2 MB (DMA engines serialize)
        HALF = SLAB // 2
        # wv halves + first slab quarters: the first v chain only needs
        # wv[dc 0..3] and xt slab0[dc 0..1], so it starts after ~0.3 MB
        WVH = N_DC * HPC * H // 2
        QTR = SLAB // 4
        for half in range(2):
            nc.gpsimd.dma_start(
                out=wv_sb[:, 4 * half : 4 * (half + 1), :],
                in_=wv[:, WVH * half : WVH * (half + 1)].rearrange(
                    "p (dc m) -> p dc m", dc=4
                ),
            )
        for q in range(4):
            nc.sync.dma_start(
                out=xt_sb[:, 0, 2 * q : 2 * (q + 1), :],
                in_=xt[:, QTR * q : QTR * (q + 1)].rearrange(
                    "p (dc c) -> p dc c", dc=2
                ),
            )
        for half in range(2):
            nc.gpsimd.dma_start(
                out=wqk_sb[:, 4 * half : 4 * (half + 1), :, :],
                in_=wqk[
                    :, 4 * 4 * 128 * half : 4 * 4 * 128 * (half + 1)
                ].rearrange("p (dc qc m) -> p dc qc m", dc=4, qc=4),
            )
        for lt in range(1, N_QT):
            nc.sync.dma_start(
                out=xt_sb[:, lt, :, :],
                in_=xt[:, SLAB * lt : SLAB * (lt + 1)].rearrange(
                    "p (dc c) -> p dc c", dc=N_DC
                ),
            )
        nc.gpsimd.dma_start(out=ctab_sb, in_=ctab[:, :])
        nc.gpsimd.dma_start(out=stab_sb, in_=stab[:, :])
        nc.gpsimd.dma_start(out=tri_sb, in_=tri[:, :])
        # w_proj rows for this core's 4 heads: chunk j holds rows for
        # heads (2j, 2j+1) stacked across the 128 partitions
        nc.gpsimd.dma_start(
            out=wp_sb[:, :, :],
            in_=wp[:, :].rearrange("p (j m) -> p j m", j=2),
        )

        # ---- v projection (normal orientation), with ones column fused ----
        # per L-chunk layout: [v_h0(64) 1 | v_h1(64) 1 | v_h2(64) 1 | v_h3(64) 1]
        v_sb = singles.tile([128, N_LC, HPC * (H + 1)], bf)
        for h in range(HPC):
            nc.vector.memset(v_sb[:, :, (H + 1) * h + H], 1.0)
        for lc in range(N_LC):
            ps = ps_proj.tile([128, HPC * H], f32, tag="proj")
            for dc in range(N_DC):
                nc.tensor.matmul(
                    ps,
                    lhsT=xt_sb[:, lc // 4, dc, 128 * (lc % 4) : 128 * (lc % 4 + 1)],
                    rhs=wv_sb[:, dc, :],
                    start=(dc == 0),
                    stop=(dc == N_DC - 1),
                )
            for h in range(HPC):
                nc.vector.tensor_copy(
                    v_sb[:, lc, (H + 1) * h : (H + 1) * h + H],
                    ps[:, H * h : H * (h + 1)],
                )

        # ---- q,k projection (transposed orientation) + RoPE ----
        # lt-outer so the first chains only need xt slab 0; one long PE
        # phase keeps the tensor engine at full p-state.
        # qk chunks: 0,1 = q heads (0,1),(2,3); 2,3 = k heads (0,1),(2,3)
        qk_roped = singles.tile([128, 4, L], bf)
        for lt in range(N_QT):
            lsl = slice(QT * lt, QT * (lt + 1))
            for qc in range(4):
                ps = ps_proj.tile([128, QT], f32, tag="proj")
                for dc in range(N_DC):
                    nc.tensor.matmul(
                        ps,
                        lhsT=wqk_sb[:, dc, qc, :],
                        rhs=xt_sb[:, lt, dc, :],
                        start=(dc == 0),
                        stop=(dc == N_DC - 1),
                    )
                qk_bf = work.tile([128, QT], bf, tag="qkbf")
                nc.scalar.activation(out=qk_bf, in_=ps, func=Copy)
                # rot[p] = qk_bf[p ^ 1]  (adjacent even/odd partner swap,
                # a within-32-partition permutation -> stream_shuffle)
                rot = work.tile([128, QT], bf, tag="rot")
                nc.vector.stream_shuffle(
                    rot, qk_bf, mask=[i ^ 1 for i in range(32)]
                )
                m1 = work.tile([128, QT], bf, tag="m1")
                nc.vector.tensor_mul(m1, qk_bf, ctab_sb[:, lsl])
                m2 = work.tile([128, QT], bf, tag="m2")
                nc.vector.tensor_mul(m2, rot, stab_sb[:, lsl])
                nc.gpsimd.tensor_add(qk_roped[:, qc, lsl], m1, m2)

        # ---- attention (scores transposed; 2-key-block groups) ----
        # Query-tile-outer so the partial output projection for tile t
        # overlaps the attention of tile t+1.  attn_all chunk j holds
        # heads (2j, 2j+1) on partition halves, matching the wp_sb row
        # layout for the final contraction.
        attn_all = singles.tile([128, 2, L], bf)

        def _out_proj_tile(tp):
            # partial output projection for tile tp's 4 L-chunks
            # (contraction over this core's 4 heads = 2 chunks of 128)
            for lc in range(4 * tp, 4 * (tp + 1)):
                lsl = slice(128 * lc, 128 * (lc + 1))
                osb = opool.tile([128, D], bf, tag="osb", name="osb")
                for oc in range(2):
                    osl = slice(QT * oc, QT * (oc + 1))
                    ps = ps_proj.tile([128, QT], f32, tag="proj", name="ps")
                    for j in range(2):
                        nc.tensor.matmul(
                            ps,
                            lhsT=attn_all[:, j, lsl],
                            rhs=wp_sb[:, j, osl],
                            start=(j == 0),
                            stop=(j == 1),
                        )
                    nc.vector.tensor_copy(osb[:, osl], ps)
                if lc % 2 == 0:
                    nc.sync.dma_start(out=out[lsl, :], in_=osb)
                else:
                    nc.gpsimd.dma_start(out=out[lsl, :], in_=osb)

        for t in range(N_QT):
            qsl = slice(QT * t, QT * (t + 1))
            for h in range(HPC):
                qc = h // 2
                kc = 2 + h // 2
                base = 64 * (h % 2)
                q_all = qk_roped[base : base + 64, qc, :]
                k_all = qk_roped[base : base + 64, kc, :]
                po = ps_pv.tile([H + 1, QT], f32, tag="pv")
                n_kb = 4 * (t + 1)
                for g in range(n_kb // 2):
                    pss = ps_scores.tile([128, 2 * QT], f32, tag="scores")
                    et = epool.tile([128, 2 * QT], bf, tag="etile")
                    for j in range(2):
                        kb = 2 * g + j
                        d = 128 * kb - QT * t  # kb/qt diagonal offset
                        lo = max(d, 0)
                        nc.tensor.matmul(
                            pss[:, QT * j + lo : QT * (j + 1)],
                            lhsT=k_all[:, 128 * kb : 128 * (kb + 1)],
                            rhs=q_all[:, QT * t + lo : QT * (t + 1)],
                            start=True,
                            stop=True,
                        )
                    # exp (with 1/sqrt(H) scale); diag blocks get separate
                    # calls restricted to their valid column range
                    if 128 * (2 * g + 1) - QT * t < 0:
                        nc.scalar.activation(
                            out=et, in_=pss, func=Exp, scale=SCALE
                        )
                    else:
                        for j in range(2):
                            kb = 2 * g + j
                            lo = max(128 * kb - QT * t, 0)
                            nc.scalar.activation(
                                out=et[:, QT * j + lo : QT * (j + 1)],
                                in_=pss[:, QT * j + lo : QT * (j + 1)],
                                func=Exp,
                                scale=SCALE,
                            )
                    for j in range(2):
                        kb = 2 * g + j
                        d = 128 * kb - QT * t
                        lo = max(d, 0)
                        if d >= -127:
                            # boundary block: zero strictly-masked entries
                            nc.vector.tensor_mul(
                                et[:, QT * j + lo : QT * j + lo + 128],
                                et[:, QT * j + lo : QT * j + lo + 128],
                                tri_sb,
                            )
                        nc.tensor.matmul(
                            po[:, lo:QT],
                            lhsT=v_sb[:, kb, (H + 1) * h : (H + 1) * (h + 1)],
                            rhs=et[:, QT * j + lo : QT * (j + 1)],
                            start=(kb == 0),
                            stop=(kb == n_kb - 1),
                        )
                # normalize: attn = po[0:64] * (1 / po[64]).  The 1/z row
                # broadcasts across 64 partitions via a DVE partition-base
                # shift (64 -> 0) + gpsimd partition_broadcast; the odd
                # head's result lands on partitions 64..127 via a DVE
                # output-side partition shift.  No DRAM round-trips.
                z0 = dpool.tile([1, QT], f32, tag="z0")
                nc.vector.reciprocal(out=z0, in_=po[H : H + 1, :])
                rb = dpool.tile([H, QT], f32, tag="rb")
                nc.gpsimd.partition_broadcast(rb, z0)
                nc.vector.tensor_mul(
                    attn_all[base : base + H, h // 2, qsl], po[0:H, :], rb
                )
                if h == 0 and t > 0:
                    # ---- partial output projection for the PREVIOUS
                    # tile, emitted after this tile's first head so the
                    # PE never waits on the previous tile's last
                    # normalize chain (DVE reciprocal -> pool broadcast
                    # -> DVE mul latency is hidden behind scores) ----
                    _out_proj_tile(t - 1)

        _out_proj_tile(N_QT - 1)

    nc.compile()
    _prog_cache["nc"] = nc
    return nc


def _host_inputs(x, rope, w_qkv, w_proj):
    """Shard + reformat the full inputs for the 8 cores."""
    rope = np.asarray(rope, dtype=np.float32)
    x = np.asarray(x, dtype=np.float32)
    w_qkv = np.asarray(w_qkv, dtype=np.float32)
    w_proj = np.asarray(w_proj, dtype=np.float32)

    # xt packed as [128, lt, dc, c]: xt[p, lt, dc, c] = x[b][lt*512+c, dc*128+p]
    xt_b = []
    for b in range(B):
        xb = x[b].T.reshape(N_DC, 128, N_QT, QT)  # [dc, p, lt, c]
        xt_b.append(
            np.ascontiguousarray(xb.transpose(1, 2, 0, 3))
            .reshape(128, N_QT * N_DC * QT)
            .astype(BF16)
        )

    # rope tables in h-major chunk layout: partition p of a 2-head chunk is
    # head (p // 64), component (p % 64); pair index i = (p % 64) // 2
    i_of_p = (np.arange(128) % 64) // 2
    cos_li = rope[:, :, 0]  # (L, 32)
    sin_li = rope[:, :, 1]
    ctab = np.ascontiguousarray(cos_li[:, i_of_p].T).astype(BF16)
    sign = np.where(np.arange(128) % 2 == 0, -1.0, 1.0).astype(np.float32)
    stab = np.ascontiguousarray((sin_li[:, i_of_p] * sign[None, :]).T).astype(BF16)

    # tri[p, f] = 1 where key offset p <= query offset f (keep), else 0
    tri = (np.arange(128)[:, None] <= np.arange(128)[None, :]).astype(BF16)

    in_maps = []
    for c in range(NCORES):
        b, g = divmod(c, GROUP)
        heads = [HPC * g + i for i in range(HPC)]
        wq = np.concatenate([w_qkv[:, H * n : H * (n + 1)] for n in heads], 1)
        wk = np.concatenate(
            [w_qkv[:, D + H * n : D + H * (n + 1)] for n in heads], 1
        )
        wvv = np.concatenate(
            [w_qkv[:, 2 * D + H * n : 2 * D + H * (n + 1)] for n in heads], 1
        )
        # wqk packed as [128, dc, qc, m]; wv as [128, dc, m]; wp as [128, j, m]
        wqk_c = np.concatenate([wq, wk], 1)  # [1024, 1024]
        wqk_p = (
            wqk_c.reshape(N_DC, 128, 4, 128)
            .transpose(1, 0, 2, 3)
            .reshape(128, N_DC * 4 * 128)
        )
        wv_p = (
            wvv.reshape(N_DC, 128, HPC * H)
            .transpose(1, 0, 2)
            .reshape(128, N_DC * HPC * H)
        )
        wp_p = (
            w_proj[WPR * g : WPR * (g + 1), :]
            .reshape(2, 128, D)
            .transpose(1, 0, 2)
            .reshape(128, 2 * D)
        )
        in_maps.append(
            {
                "xt": xt_b[b],
                "wqk": np.ascontiguousarray(wqk_p).astype(BF16),
                "wv": np.ascontiguousarray(wv_p).astype(BF16),
                "wp": np.ascontiguousarray(wp_p).astype(BF16),
                "ctab": ctab,
                "stab": stab,
                "tri": tri,
            }
        )
    return in_maps


def kernel(x, rope, mask, w_qkv, w_proj, _trace=False):
    from concourse.bass_utils import run_bass_kernel_spmd

    nc = _build_program()
    in_maps = _host_inputs(x, rope, w_qkv, w_proj)
    res = run_bass_kernel_spmd(
        nc, in_maps, core_ids=list(range(NCORES)), trace=_trace
    )
    _prog_cache["last_result"] = res

    full = np.empty((B, L, D), dtype=np.float32)
    for b in range(B):
        acc = np.zeros((L, D), dtype=np.float32)
        for g in range(GROUP):
            acc += np.asarray(res.results[GROUP * b + g]["out"], dtype=np.float32)
        full[b] = acc
    return full


# revision 36
# speedup vs baseline: 1.3098x; 1.0035x over previous
"""Trainium2 Bass kernel for causal self-attention with RoPE.

Problem shapes (hardcoded): B=2, L=2048, D=1024, N=16 heads, H=64.

Sharding (8 cores, fully collective-free): data-parallel over batch
(2 groups of 4 cores), tensor-parallel over heads within a group
(4 heads/core).  Each core:
  1. computes q,k for its 4 heads in h-major layout (transposed matmul
     orientation: lhsT = w columns, rhs = x^T), applies RoPE on-chip,
  2. computes v in L-major layout (normal orientation),
  3. runs causal flash-style attention with scores transposed
     (S^T[key, query]) so softmax sums ride a fused ones-column through
     the PV matmul (no transposes anywhere),
  4. computes its PARTIAL output projection: its 4 heads' attention
     outputs (256 contraction dims) times the matching 256-row slice of
     w_proj, giving a full [L, D] partial in bf16.
Host code reformats/shards inputs (transpose, bf16 cast, column
permutation, table replication) and sums the 4 partials per batch.
There is no cross-core communication anywhere, so each core's
execution span is independent of peer launch skew.
"""

import numpy as np
import ml_dtypes

B, L, D, N_HEADS, H = 2, 2048, 1024, 16, 64
HPC = 4          # heads per core
GROUP = 4        # cores per batch group
NCORES = 8
QT = 512         # query tile width (matmul free dim)
KB = 128         # key block (psum partition dim)
N_QT = L // QT   # 4 query tiles
N_DC = D // 128  # 8 contraction chunks
N_LC = L // 128  # 16 L chunks for v / output rows
WPR = HPC * H    # w_proj rows per core (256)
BF16 = ml_dtypes.bfloat16

_prog_cache = {}


def _build_program():
    if "nc" in _prog_cache:
        return _prog_cache["nc"]

    import concourse.bass as bass
    import concourse.mybir as mybir
    import concourse.tile as tile
    from concourse import bacc
    from contextlib import ExitStack

    bf = mybir.dt.bfloat16
    f32 = mybir.dt.float32

    nc = bacc.Bacc(num_devices=NCORES)

    # host pre-packs everything into the SBUF layout: [128, ...free dims]
    xt = nc.dram_tensor("xt", [128, N_QT * N_DC * QT], bf, kind="ExternalInput")
    wqk = nc.dram_tensor("wqk", [128, N_DC * 4 * 128], bf, kind="ExternalInput")
    wv = nc.dram_tensor("wv", [128, N_DC * HPC * H], bf, kind="ExternalInput")
    wp = nc.dram_tensor("wp", [128, 2 * D], bf, kind="ExternalInput")
    ctab = nc.dram_tensor("ctab", [128, L], bf, kind="ExternalInput")
    stab = nc.dram_tensor("stab", [128, L], bf, kind="ExternalInput")
    tri = nc.dram_tensor("tri", [128, 128], bf, kind="ExternalInput")
    out = nc.dram_tensor("out", [L, D], bf, kind="ExternalOutput")

    Exp = mybir.ActivationFunctionType.Exp
    Copy = mybir.ActivationFunctionType.Copy
    SCALE = 1.0 / 8.0  # 1/sqrt(H)

    with tile.TileContext(nc) as tc, ExitStack() as ctx:
        singles = ctx.enter_context(tc.tile_pool(name="singles", bufs=1))
        work = ctx.enter_context(tc.tile_pool(name="work", bufs=6))
        epool = ctx.enter_context(tc.tile_pool(name="epool", bufs=6))
        dpool = ctx.enter_context(tc.tile_pool(name="dpool", bufs=4))
        opool = ctx.enter_context(tc.tile_pool(name="opool", bufs=4))
        ps_scores = ctx.enter_context(
            tc.tile_pool(name="ps_scores", bufs=2, space="PSUM")
        )
        ps_pv = ctx.enter_context(tc.tile_pool(name="ps_pv", bufs=2, space="PSUM"))
        ps_proj = ctx.enter_context(
            tc.tile_pool(name="ps_proj", bufs=2, space="PSUM")
        )

        # ---- load inputs to SBUF.  Host pre-packs every tensor into its
        # SBUF-resident layout so each load is ONE dma_start with big
        # contiguous per-partition runs: xt arrives as 4 query-tile slabs
        # [128, dc, 512] so the first q,k matmul chain starts after ~1 MB
        # lands; weights/tables are single loads on the pool queue. ----
        xt_sb = singles.tile([128, N_QT, N_DC, QT], bf)
        wqk_sb = singles.tile([128, N_DC, 4, 128], bf)
        wv_sb = singles.tile([128, N_DC, HPC * H], bf)
        wp_sb = singles.tile([128, 2, D], bf)
        ctab_sb = singles.tile([128, L], bf)
        stab_sb = singles.tile([128, L], bf)
        tri_sb = singles.tile([128, 128], bf)
        SLAB = N_DC * QT
        # first slab + wqk split into dc-halves so the first q,k chain
        # starts after ~0.5 MB instead of 2 MB (DMA engines serialize)
        HALF = SLAB // 2
        # wv halves + first slab quarters: the first v chain only needs
        # wv[dc 0..3] and xt slab0[dc 0..1], so it starts after ~0.3 MB
        WVH = N_DC * HPC * H // 2
        QTR = SLAB // 4
        for half in range(2):
            nc.gpsimd.dma_start(
                out=wv_sb[:, 4 * half : 4 * (half + 1), :],
                in_=wv[:, WVH * half : WVH * (half + 1)].rearrange(
                    "p (dc m) -> p dc m", dc=4
                ),
            )
        for q in range(4):
            nc.sync.dma_start(
                out=xt_sb[:, 0, 2 * q : 2 * (q + 1), :],
                in_=xt[:, QTR * q : QTR * (q + 1)].rearrange(
                    "p (dc c) -> p dc c", dc=2
                ),
            )
        for half in range(2):
            nc.gpsimd.dma_start(
                out=wqk_sb[:, 4 * half : 4 * (half + 1), :, :],
                in_=wqk[
                    :, 4 * 4 * 128 * half : 4 * 4 * 128 * (half + 1)
                ].rearrange("p (dc qc m) -> p dc qc m", dc=4, qc=4),
            )
        for lt in range(1, N_QT):
            nc.sync.dma_start(
                out=xt_sb[:, lt, :, :],
                in_=xt[:, SLAB * lt : SLAB * (lt + 1)].rearrange(
                    "p (dc c) -> p dc c", dc=N_DC
                ),
            )
        nc.gpsimd.dma_start(out=ctab_sb, in_=ctab[:, :])
        nc.gpsimd.dma_start(out=stab_sb, in_=stab[:, :])
        nc.gpsimd.dma_start(out=tri_sb, in_=tri[:, :])
        # w_proj rows for this core's 4 heads: chunk j holds rows for
        # heads (2j, 2j+1) stacked across the 128 partitions
        nc.gpsimd.dma_start(
            out=wp_sb[:, :, :],
            in_=wp[:, :].rearrange("p (j m) -> p j m", j=2),
        )

        # ---- v projection (normal orientation), with ones column fused ----
        # per L-chunk layout: [v_h0(64) 1 | v_h1(64) 1 | v_h2(64) 1 | v_h3(64) 1]
        v_sb = singles.tile([128, N_LC, HPC * (H + 1)], bf)
        for h in range(HPC):
            nc.vector.memset(v_sb[:, :, (H + 1) * h + H], 1.0)
        for lc in range(N_LC):
            ps = ps_proj.tile([128, HPC * H], f32, tag="proj")
            for dc in range(N_DC):
                nc.tensor.matmul(
                    ps,
                    lhsT=xt_sb[:, lc // 4, dc, 128 * (lc % 4) : 128 * (lc % 4 + 1)],
                    rhs=wv_sb[:, dc, :],
                    start=(dc == 0),
                    stop=(dc == N_DC - 1),
                )
            for h in range(HPC):
                nc.vector.tensor_copy(
                    v_sb[:, lc, (H + 1) * h : (H + 1) * h + H],
                    ps[:, H * h : H * (h + 1)],
                )

        # ---- q,k projection (transposed orientation) + RoPE ----
        # lt-outer so the first chains only need xt slab 0; one long PE
        # phase keeps the tensor engine at full p-state.
        # qk chunks: 0,1 = q heads (0,1),(2,3); 2,3 = k heads (0,1),(2,3)
        qk_roped = singles.tile([128, 4, L], bf)
        for lt in range(N_QT):
            lsl = slice(QT * lt, QT * (lt + 1))
            for qc in range(4):
                ps = ps_proj.tile([128, QT], f32, tag="proj")
                for dc in range(N_DC):
                    nc.tensor.matmul(
                        ps,
                        lhsT=wqk_sb[:, dc, qc, :],
                        rhs=xt_sb[:, lt, dc, :],
                        start=(dc == 0),
                        stop=(dc == N_DC - 1),
                    )
                qk_bf = work.tile([128, QT], bf, tag="qkbf")
                nc.scalar.activation(out=qk_bf, in_=ps, func=Copy)
                # rot[p] = qk_bf[p ^ 1]  (adjacent even/odd partner swap,
                # a within-32-partition permutation -> stream_shuffle)
                rot = work.tile([128, QT], bf, tag="rot")
                nc.vector.stream_shuffle(
                    rot, qk_bf, mask=[i ^ 1 for i in range(32)]
                )
                m1 = work.tile([128, QT], bf, tag="m1")
                nc.vector.tensor_mul(m1, qk_bf, ctab_sb[:, lsl])
                m2 = work.tile([128, QT], bf, tag="m2")
                nc.vector.tensor_mul(m2, rot, stab_sb[:, lsl])
                nc.gpsimd.tensor_add(qk_roped[:, qc, lsl], m1, m2)

        # ---- attention (scores transposed; 2-key-block groups) ----
        # Query-tile-outer so the partial output projection for tile t
        # overlaps the attention of tile t+1.  attn_all chunk j holds
        # heads (2j, 2j+1) on partition halves, matching the wp_sb row
        # layout for the final contraction.
        attn_all = singles.tile([128, 2, L], bf)

        def _out_proj_tile(tp):
            # partial output projection for tile tp's 4 L-chunks
            # (contraction over this core's 4 heads = 2 chunks of 128).
            # The last tile writes per column-half so the final DMAs
            # start as soon as each copy lands (shorter drain tail).
            last = tp == N_QT - 1
            for lc in range(4 * tp, 4 * (tp + 1)):
                lsl = slice(128 * lc, 128 * (lc + 1))
                osb = opool.tile([128, D], bf, tag="osb", name="osb")
                for oc in range(2):
                    osl = slice(QT * oc, QT * (oc + 1))
                    ps = ps_proj.tile([128, QT], f32, tag="proj", name="ps")
                    for j in range(2):
                        nc.tensor.matmul(
                            ps,
                            lhsT=attn_all[:, j, lsl],
                            rhs=wp_sb[:, j, osl],
                            start=(j == 0),
                            stop=(j == 1),
                        )
                    nc.vector.tensor_copy(osb[:, osl], ps)
                    if last:
                        eng = nc.sync if oc == 0 else nc.gpsimd
                        eng.dma_start(out=out[lsl, osl], in_=osb[:, osl])
                if not last:
                    if lc % 2 == 0:
                        nc.sync.dma_start(out=out[lsl, :], in_=osb)
                    else:
                        nc.gpsimd.dma_start(out=out[lsl, :], in_=osb)

        for t in range(N_QT):
            qsl = slice(QT * t, QT * (t + 1))
            for h in range(HPC):
                qc = h // 2
                kc = 2 + h // 2
                base = 64 * (h % 2)
                q_all = qk_roped[base : base + 64, qc, :]
                k_all = qk_roped[base : base + 64, kc, :]
                po = ps_pv.tile([H + 1, QT], f32, tag="pv")
                n_kb = 4 * (t + 1)
                for g in range(n_kb // 2):
                    pss = ps_scores.tile([128, 2 * QT], f32, tag="scores")
                    et = epool.tile([128, 2 * QT], bf, tag="etile")
                    for j in range(2):
                        kb = 2 * g + j
                        d = 128 * kb - QT * t  # kb/qt diagonal offset
                        lo = max(d, 0)
                        nc.tensor.matmul(
                            pss[:, QT * j + lo : QT * (j + 1)],
                            lhsT=k_all[:, 128 * kb : 128 * (kb + 1)],
                            rhs=q_all[:, QT * t + lo : QT * (t + 1)],
                            start=True,
                            stop=True,
                        )
                    # exp (with 1/sqrt(H) scale); diag blocks get separate
                    # calls restricted to their valid column range
                    if 128 * (2 * g + 1) - QT * t < 0:
                        nc.scalar.activation(
                            out=et, in_=pss, func=Exp, scale=SCALE
                        )
                    else:
                        for j in range(2):
                            kb = 2 * g + j
                            lo = max(128 * kb - QT * t, 0)
                            nc.scalar.activation(
                                out=et[:, QT * j + lo : QT * (j + 1)],
                                in_=pss[:, QT * j + lo : QT * (j + 1)],
                                func=Exp,
                                scale=SCALE,
                            )
                    for j in range(2):
                        kb = 2 * g + j
                        d = 128 * kb - QT * t
                        lo = max(d, 0)
                        if d >= -127:
                            # boundary block: zero strictly-masked entries
                            nc.vector.tensor_mul(
                                et[:, QT * j + lo : QT * j + lo + 128],
                                et[:, QT * j + lo : QT * j + lo + 128],
                                tri_sb,
                            )
                        nc.tensor.matmul(
                            po[:, lo:QT],
                            lhsT=v_sb[:, kb, (H + 1) * h : (H + 1) * (h + 1)],
                            rhs=et[:, QT * j + lo : QT * (j + 1)],
                            start=(kb == 0),
                            stop=(kb == n_kb - 1),
                        )
                # normalize: attn = po[0:64] * (1 / po[64]).  The 1/z row
                # broadcasts across 64 partitions via a DVE partition-base
                # shift (64 -> 0) + gpsimd partition_broadcast; the odd
                # head's result lands on partitions 64..127 via a DVE
                # output-side partition shift.  No DRAM round-trips.
                z0 = dpool.tile([1, QT], f32, tag="z0")
                nc.vector.reciprocal(out=z0, in_=po[H : H + 1, :])
                rb = dpool.tile([H, QT], f32, tag="rb")
                nc.gpsimd.partition_broadcast(rb, z0)
                nc.vector.tensor_mul(
                    attn_all[base : base + H, h // 2, qsl], po[0:H, :], rb
                )
                if h == 0 and t > 0:
                    # ---- partial output projection for the PREVIOUS
                    # tile, emitted after this tile's first head so the
                    # PE never waits on the previous tile's last
                    # normalize chain (DVE reciprocal -> pool broadcast
                    # -> DVE mul latency is hidden behind scores) ----
                    _out_proj_tile(t - 1)

        _out_proj_tile(N_QT - 1)

    nc.compile()
    _prog_cache["nc"] = nc
    return nc


def _host_inputs(x, rope, w_qkv, w_proj):
    """Shard + reformat the full inputs for the 8 cores."""
    rope = np.asarray(rope, dtype=np.float32)
    x = np.asarray(x, dtype=np.float32)
    w_qkv = np.asarray(w_qkv, dtype=np.float32)
    w_proj = np.asarray(w_proj, dtype=np.float32)

    # xt packed as [128, lt, dc, c]: xt[p, lt, dc, c] = x[b][lt*512+c, dc*128+p]
    xt_b = []
    for b in range(B):
        xb = x[b].T.reshape(N_DC, 128, N_QT, QT)  # [dc, p, lt, c]
        xt_b.append(
            np.ascontiguousarray(xb.transpose(1, 2, 0, 3))
            .reshape(128, N_QT * N_DC * QT)
            .astype(BF16)
        )

    # rope tables in h-major chunk layout: partition p of a 2-head chunk is
    # head (p // 64), component (p % 64); pair index i = (p % 64) // 2
    i_of_p = (np.arange(128) % 64) // 2
    cos_li = rope[:, :, 0]  # (L, 32)
    sin_li = rope[:, :, 1]
    ctab = np.ascontiguousarray(cos_li[:, i_of_p].T).astype(BF16)
    sign = np.where(np.arange(128) % 2 == 0, -1.0, 1.0).astype(np.float32)
    stab = np.ascontiguousarray((sin_li[:, i_of_p] * sign[None, :]).T).astype(BF16)

    # tri[p, f] = 1 where key offset p <= query offset f (keep), else 0
    tri = (np.arange(128)[:, None] <= np.arange(128)[None, :]).astype(BF16)

    in_maps = []
    for c in range(NCORES):
        b, g = divmod(c, GROUP)
        heads = [HPC * g + i for i in range(HPC)]
        wq = np.concatenate([w_qkv[:, H * n : H * (n + 1)] for n in heads], 1)
        wk = np.concatenate(
            [w_qkv[:, D + H * n : D + H * (n + 1)] for n in heads], 1
        )
        wvv = np.concatenate(
            [w_qkv[:, 2 * D + H * n : 2 * D + H * (n + 1)] for n in heads], 1
        )
        # wqk packed as [128, dc, qc, m]; wv as [128, dc, m]; wp as [128, j, m]
        wqk_c = np.concatenate([wq, wk], 1)  # [1024, 1024]
        wqk_p = (
            wqk_c.reshape(N_DC, 128, 4, 128)
            .transpose(1, 0, 2, 3)
            .reshape(128, N_DC * 4 * 128)
        )
        wv_p = (
            wvv.reshape(N_DC, 128, HPC * H)
            .transpose(1, 0, 2)
            .reshape(128, N_DC * HPC * H)
        )
        wp_p = (
            w_proj[WPR * g : WPR * (g + 1), :]
            .reshape(2, 128, D)
            .transpose(1, 0, 2)
            .reshape(128, 2 * D)
        )
        in_maps.append(
            {
                "xt": xt_b[b],
                "wqk": np.ascontiguousarray(wqk_p).astype(BF16),
                "wv": np.ascontiguousarray(wv_p).astype(BF16),
                "wp": np.ascontiguousarray(wp_p).astype(BF16),
                "ctab": ctab,
                "stab": stab,
                "tri": tri,
            }
        )
    return in_maps


def kernel(x, rope, mask, w_qkv, w_proj, _trace=False):
    from concourse.bass_utils import run_bass_kernel_spmd

    nc = _build_program()
    in_maps = _host_inputs(x, rope, w_qkv, w_proj)
    res = run_bass_kernel_spmd(
        nc, in_maps, core_ids=list(range(NCORES)), trace=_trace
    )
    _prog_cache["last_result"] = res

    full = np.empty((B, L, D), dtype=np.float32)
    for b in range(B):
        acc = np.zeros((L, D), dtype=np.float32)
        for g in range(GROUP):
            acc += np.asarray(res.results[GROUP * b + g]["out"], dtype=np.float32)
        full[b] = acc
    return full
